# revision 1
# baseline (speedup 1.0000x reference)
"""MoE FFN (capacity-routed, top-2, SwiGLU) on 8 TRN2 NeuronCores.

Expert-parallel: one expert per core.  Router (RMSNorm + gate + top-2) is
token-sharded (512 tokens/core) and all-gathered; dispatch/combine are
realized as indirect DMA gather / scatter-add driven by on-device
position (cumsum) computation that reproduces the reference's
per-(expert, k-slot) capacity stream semantics exactly.  Final combine
reduction across experts is a ReduceScatter; host concatenates slices.
"""

import numpy as np

E, K, D, H = 8, 2, 1024, 4096
B, S = 2, 2048
T = B * S                      # 4096
TPC = T // 8                   # 512 tokens per core
CAP = int(1.5 * T * K / E)     # 1536
RMS_EPS = 1e-6
ROW = 1152                     # padded xn row: 1024 x | p0 | p1 | pad  (2304B % 256 == 0)
NS = 3                         # 1536 slots = 3 slices of 512
WRAP = CAP // 16               # 96 wrapped idx cols


def build_bass():
    import concourse.bass as bass
    import concourse.mybir as mybir
    from concourse import bacc, tile

    f32 = mybir.dt.float32
    bf16 = mybir.dt.bfloat16
    i16 = mybir.dt.int16
    u32 = mybir.dt.uint32
    AF = mybir.ActivationFunctionType
    OP = mybir.AluOpType
    AX = mybir.AxisListType
    ts = bass.ts

    nc = bacc.Bacc("TRN2", target_bir_lowering=False, debug=False, num_devices=8)

    xs = nc.dram_tensor("xs", [TPC, D], f32, kind="ExternalInput").ap()
    gw = nc.dram_tensor("gw", [D, E], f32, kind="ExternalInput").ap()
    w1b = nc.dram_tensor("w1b", [128, 32, 8, 128], bf16, kind="ExternalInput").ap()
    w2b = nc.dram_tensor("w2b", [128, 32, 8, 128], bf16, kind="ExternalInput").ap()
    w3b = nc.dram_tensor("w3b", [8, 128, 4, D], bf16, kind="ExternalInput").ap()
    eid = nc.dram_tensor("eid", [16, 1], f32, kind="ExternalInput").ap()
    ident = nc.dram_tensor("ident", [128, 128], f32, kind="ExternalInput").ap()
    out = nc.dram_tensor("out", [TPC, D], f32, kind="ExternalOutput").ap()

    RG = [list(range(8))]

    with tile.TileContext(nc) as tc:
        with (
            tc.tile_pool(name="dram", bufs=1, space="DRAM") as dp,
            tc.tile_pool(name="const", bufs=1) as cst,
            tc.tile_pool(name="lists", bufs=1) as lp,
            tc.tile_pool(name="eip", bufs=1) as eip,
        ):
            # ---- internal DRAM ----
            xn_loc = dp.tile([TPC + 16, ROW], bf16)
            tk_loc = dp.tile([2, TPC], f32)
            xn_full = dp.tile([(TPC + 16) * 8, ROW], bf16, addr_space="Shared")
            tk_full = dp.tile([8, 2, TPC], f32, addr_space="Shared")
            partial = dp.tile([T, D], f32)
            rs_out = dp.tile([TPC, D], f32)
            sl_dram = dp.tile([2, CAP], i16)
            gl_dram = dp.tile([2, CAP], i16)
            gate_dram = dp.tile([2, CAP], bf16)

            # ---- constants ----
            id_sb = cst.tile([128, 128], f32)
            nc.sync.dma_start(id_sb[:], ident)
            gw_sb = cst.tile([128, 8, E], f32)
            nc.sync.dma_start(gw_sb[:], gw.rearrange("(dc p) e -> p dc e", p=128))
            eid_sb = cst.tile([16, 1], f32)
            nc.sync.dma_start(eid_sb[:], eid)
            eps_col = cst.tile([128, 1], f32)
            nc.vector.memset(eps_col[:], RMS_EPS)

            # ---- zero-fill partial + xn_full zero row ----
            zf = cst.tile([128, D], f32)
            nc.vector.memset(zf[:], 0.0)
            for i in range(T // 128):
                nc.scalar.dma_start(partial[ts(i, 128), :], zf[:])
            zpad = cst.tile([16, ROW], bf16)
            nc.vector.memset(zpad[:], 0.0)
            nc.scalar.dma_start(xn_loc[TPC : TPC + 16, :], zpad[:])

            # ---- long-lived small tiles ----
            # idx lists live in (128, n/16) tiles: HW/sim read the wrapped
            # pattern from partitions 0-15; rows 16-127 are zero filler.
            slw = [lp.tile([128, WRAP], i16, name=f"slw{k}") for k in range(2)]
            glw = [lp.tile([128, WRAP], i16, name=f"glw{k}") for k in range(2)]
            ntile = lp.tile([2, 12], mybir.dt.int32, name="ntile")
            gates = [lp.tile([128, NS * 4], bf16, name=f"gates{k}") for k in range(2)]
            gatesf = [lp.tile([128, NS * 4], f32, name=f"gatesf{k}") for k in range(2)]
            ei = eip.tile([128, 8, CAP], bf16)

            # ================= router (local 512 tokens) =================
            with (
                tc.tile_pool(name="rout", bufs=2) as rp,
                tc.tile_pool(name="routc", bufs=4) as rc,
                tc.tile_pool(name="rpsum", bufs=2, space="PSUM") as rps,
            ):
                for i in range(TPC // 128):
                    xt = rp.tile([128, D], f32, tag="xt")
                    nc.sync.dma_start(xt[:], xs[ts(i, 128), :])
                    sq = rps.tile([128, D], f32, tag="sq")
                    ssum = rc.tile([128, 1], f32, tag="ssum")
                    nc.scalar.activation(sq[:], xt[:], AF.Square, accum_out=ssum[:])
                    s1 = rc.tile([128, 1], f32, tag="s1")
                    nc.scalar.activation(
                        s1[:], ssum[:], AF.Sqrt, bias=eps_col[:], scale=1.0 / D
                    )
                    r1 = rc.tile([128, 1], f32, tag="r1")
                    nc.vector.reciprocal(r1[:], s1[:])
                    xnf = rp.tile([128, D], f32, tag="xnf")
                    nc.scalar.activation(xnf[:], xt[:], AF.Copy, scale=r1[:])
                    xnb = rp.tile([128, D], bf16, tag="xnb")
                    nc.vector.tensor_copy(xnb[:], xnf[:])
                    nc.sync.dma_start(xn_loc[ts(i, 128), 0:D], xnb[:])

                    # transpose x_norm tile, then logits = xnT.T @ gw -> (tok, E)
                    xnT = rp.tile([128, 8, 128], f32, tag="xnT")
                    for dc in range(8):
                        tp = rps.tile([128, 128], f32, tag="tp")
                        nc.tensor.transpose(tp[:], xnf[:, ts(dc, 128)], id_sb[:])
                        nc.scalar.copy(xnT[:, dc, :], tp[:])
                    lps = rps.tile([128, E], f32, tag="lps")
                    for dc in range(8):
                        nc.tensor.matmul(
                            lps[:], xnT[:, dc, :], gw_sb[:, dc, :],
                            start=(dc == 0), stop=(dc == 7),
                        )
                    lg = rp.tile([128, E], f32, tag="lg")
                    nc.vector.tensor_copy(lg[:], lps[:])

                    mx = rp.tile([128, 8], f32, tag="mx")
                    nc.vector.max(mx[:], lg[:])
                    mi = rp.tile([128, 8], u32, tag="mi")
                    nc.vector.max_index(mi[:], mx[:], lg[:])

                    negm1 = rc.tile([128, 1], f32, tag="negm1")
                    nc.vector.tensor_scalar_mul(negm1[:], mx[:, 0:1], -1.0)
                    ex = rp.tile([128, E], f32, tag="ex")
                    nc.scalar.activation(ex[:], lg[:], AF.Exp, bias=negm1[:])
                    zz = rc.tile([128, 1], f32, tag="zz")
                    nc.vector.reduce_sum(zz[:], ex[:], axis=AX.X)
                    t2 = rc.tile([128, 1], f32, tag="t2")
                    nc.scalar.activation(t2[:], mx[:, 1:2], AF.Exp, bias=negm1[:])
                    u0 = rc.tile([128, 1], f32, tag="u0")
                    nc.vector.scalar_tensor_tensor(
                        u0[:], zz[:], 1e-10, t2[:], op0=OP.mult, op1=OP.add
                    )
                    u1 = rc.tile([128, 1], f32, tag="u1")
                    nc.vector.tensor_scalar_add(u1[:], u0[:], 1.0)
                    p1 = rc.tile([128, 1], f32, tag="p1")
                    nc.vector.reciprocal(p1[:], u1[:])
                    p2 = rc.tile([128, 1], f32, tag="p2")
                    nc.vector.tensor_mul(p2[:], t2[:], p1[:])

                    idxf = rp.tile([128, 2], f32, tag="idxf")
                    nc.vector.tensor_copy(idxf[:], mi[:, 0:2])
                    nc.scalar.dma_start(tk_loc[0:1, ts(i, 128)], idxf[:, 0:1])
                    nc.scalar.dma_start(tk_loc[1:2, ts(i, 128)], idxf[:, 1:2])

                    p1b = rc.tile([128, 1], bf16, tag="p1b")
                    nc.vector.tensor_copy(p1b[:], p1[:])
                    p2b = rc.tile([128, 1], bf16, tag="p2b")
                    nc.vector.tensor_copy(p2b[:], p2[:])
                    nc.scalar.dma_start(xn_loc[ts(i, 128), D : D + 1], p1b[:])
                    nc.scalar.dma_start(xn_loc[ts(i, 128), D + 1 : D + 2], p2b[:])

            # ================= all-gathers =================
            nc.gpsimd.collective_compute(
                "AllGather", OP.bypass, RG, ins=[xn_loc.opt()],
                outs=[xn_full.opt()],
            )
            nc.gpsimd.collective_compute(
                "AllGather", OP.bypass, RG, ins=[tk_loc.opt()],
                outs=[tk_full.opt()],
            )

            # ================= positions / slot lists =================
            with tc.tile_pool(name="comp", bufs=1) as cp:
                idxr = cp.tile([16, T], f32)
                for b in range(8):
                    eng = nc.sync if b % 2 == 0 else nc.scalar
                    eng.dma_start(
                        idxr[2 * b : 2 * b + 2, :],
                        tk_full.rearrange("r f t -> f r t"),
                    )
                mask = cp.tile([16, T], f32)
                nc.vector.tensor_scalar(
                    out=mask[:], in0=idxr[:], scalar1=eid_sb[:], scalar2=None,
                    op0=OP.is_equal,
                )
                zer16 = cp.tile([16, T], f32)
                nc.vector.memset(zer16[:], 0.0)
                cum = cp.tile([16, T], f32)
                nc.vector.tensor_tensor_scan(
                    cum[:], mask[:], zer16[:], 0.0, op0=OP.add, op1=OP.add
                )
                # per-(stream, slot-tile) valid counts for scatter descriptors
                cnt = cp.tile([2, 1], f32)
                nc.vector.reduce_sum(cnt[:], mask[0:2, :], axis=AX.X)
                nc.vector.tensor_scalar_min(cnt[:], cnt[:], float(CAP))
                srow = cp.tile([2, 12], f32)
                nc.gpsimd.iota(
                    srow[:], pattern=[[-128, 12]], base=0, channel_multiplier=0,
                    allow_small_or_imprecise_dtypes=True,
                )
                ntf = cp.tile([2, 12], f32)
                nc.vector.tensor_scalar(
                    out=ntf[:], in0=srow[:], scalar1=cnt[:], scalar2=None,
                    op0=OP.add,
                )
                nc.vector.tensor_scalar_min(ntf[:], ntf[:], 128.0)
                nc.vector.tensor_scalar_max(ntf[:], ntf[:], 0.0)
                nc.vector.tensor_copy(ntile[:], ntf[:])

                nc.vector.tensor_tensor(
                    out=cum[:], in0=cum[:], in1=mask[:], op=OP.mult
                )
                pos16 = cp.tile([16, T], i16)
                nc.vector.tensor_scalar(
                    out=pos16[:], in0=cum[:], scalar1=-1.0, scalar2=None,
                    op0=OP.add,
                )
                tok16 = cp.tile([16, T], i16)
                nc.gpsimd.iota(
                    tok16[:], pattern=[[1, T]], base=1, channel_multiplier=0
                )
                sraw = cp.tile([16, 2046], i16)
                nc.gpsimd.local_scatter(
                    sraw[:], tok16[:], pos16[:], channels=16, num_elems=2046,
                    num_idxs=T,
                )
                # second scatter carries the 528-block gather row index:
                # iota value = 1 + b*528 + j for token t = b*512 + j
                tokg = cp.tile([16, T], i16)
                nc.gpsimd.iota(
                    tokg[:], pattern=[[TPC + 16, 8], [1, TPC]], base=1,
                    channel_multiplier=0,
                )
                sraw_g = cp.tile([16, 2046], i16)
                nc.gpsimd.local_scatter(
                    sraw_g[:], tokg[:], pos16[:], channels=16, num_elems=2046,
                    num_idxs=T,
                )
                sl = cp.tile([16, CAP], i16)
                nc.vector.tensor_scalar(
                    out=sl[:], in0=sraw[:, 0:CAP], scalar1=-1, scalar2=None,
                    op0=OP.add,
                )
                em = cp.tile([16, CAP], i16)
                nc.vector.tensor_scalar(
                    out=em[:], in0=sraw_g[:, 0:CAP], scalar1=0, scalar2=None,
                    op0=OP.is_equal,
                )
                gl = cp.tile([16, CAP], i16)
                nc.vector.tensor_scalar(
                    out=gl[:], in0=sraw_g[:, 0:CAP], scalar1=-1, scalar2=None,
                    op0=OP.add,
                )
                nc.vector.scalar_tensor_tensor(
                    gl[:], em[:], TPC + 1, gl[:], op0=OP.mult, op1=OP.add
                )
                nc.sync.dma_start(sl_dram[:, :], sl[0:2, :])
                nc.scalar.dma_start(gl_dram[:, :], gl[0:2, :])
                for k in range(2):
                    for b in range(8):
                        eng = nc.sync if b % 2 == 0 else nc.scalar
                        eng.dma_start(
                            slw[k][16 * b : 16 * (b + 1), :],
                            sl_dram[k, :].rearrange("(f p) -> p f", p=16),
                        )
                        eng.dma_start(
                            glw[k][16 * b : 16 * (b + 1), :],
                            gl_dram[k, :].rearrange("(f p) -> p f", p=16),
                        )

            # ================= token gather =================
            with tc.tile_pool(name="gath", bufs=2) as gp:
                for ns in range(NS):
                    gc = []
                    for k in range(2):
                        g = gp.tile([128, 9, 512], bf16, tag=f"g{k}", name=f"g{k}_{ns}")
                        nc.gpsimd.dma_gather(
                            g[:], xn_full[:, :], glw[k][:, ns * 32 : (ns + 1) * 32],
                            num_idxs=512, num_idxs_reg=512, elem_size=ROW,
                            transpose=True,
                        )
                        gc.append(g)
                    nc.vector.tensor_tensor(
                        out=ei[:, :, ts(ns, 512)], in0=gc[0][:, 0:8, :],
                        in1=gc[1][:, 0:8, :], op=OP.add,
                    )
                    for k in range(2):
                        nc.scalar.dma_start(
                            gate_dram[k, ts(ns, 512)], gc[k][k : k + 1, 8, :]
                        )
                for k in range(2):
                    nc.scalar.dma_start(
                        gates[k][:], gate_dram[k, :].rearrange("(f p) -> p f", p=128)
                    )
                    nc.vector.tensor_copy(gatesf[k][:], gates[k][:])

            # ================= expert FFN + combine =================
            with (
                tc.tile_pool(name="wts12", bufs=2) as wp,
                tc.tile_pool(name="wts3", bufs=6) as wp3,
                tc.tile_pool(name="hidp", bufs=1) as hp,
                tc.tile_pool(name="silp", bufs=2) as sp,
                tc.tile_pool(name="scp", bufs=2) as scp,
                tc.tile_pool(name="ps1", bufs=2, space="PSUM") as pp1,
                tc.tile_pool(name="ps2", bufs=1, space="PSUM") as pp2,
            ):
                for ns in range(NS):
                    hid = hp.tile([128, 32, 512], bf16, tag="hid", name=f"hid{ns}")
                    for mg in range(8):
                        w1t = wp.tile([128, 4, 8, 128], bf16, tag="w1", name=f"w1_{ns}_{mg}")
                        nc.sync.dma_start(w1t[:], w1b[:, mg * 4 : (mg + 1) * 4, :, :])
                        w2t = wp.tile([128, 4, 8, 128], bf16, tag="w2", name=f"w2_{ns}_{mg}")
                        nc.sync.dma_start(w2t[:], w2b[:, mg * 4 : (mg + 1) * 4, :, :])
                        for mj in range(4):
                            m = mg * 4 + mj
                            ph1 = pp1.tile([128, 512], f32, tag="ph1", name=f"ph1_{ns}_{m}")
                            ph2 = pp1.tile([128, 512], f32, tag="ph2", name=f"ph2_{ns}_{m}")
                            for dc in range(8):
                                nc.tensor.matmul(
                                    ph1[:], w1t[:, mj, dc, :], ei[:, dc, ts(ns, 512)],
                                    start=(dc == 0), stop=(dc == 7),
                                )
                            for dc in range(8):
                                nc.tensor.matmul(
                                    ph2[:], w2t[:, mj, dc, :], ei[:, dc, ts(ns, 512)],
                                    start=(dc == 0), stop=(dc == 7),
                                )
                            slt = sp.tile([128, 512], bf16, tag="sl", name=f"sl_{ns}_{m}")
                            nc.scalar.activation(slt[:], ph1[:], AF.Sigmoid)
                            tt = sp.tile([128, 512], bf16, tag="tt", name=f"tt_{ns}_{m}")
                            nc.vector.tensor_mul(tt[:], slt[:], ph1[:])
                            nc.vector.tensor_mul(hid[:, m, :], tt[:], ph2[:])

                    for sg in range(2):
                        eo = [
                            pp2.tile([128, D], f32, tag=f"eo{j}", name=f"eo_{ns}_{sg}_{j}")
                            for j in range(2)
                        ]
                        for hg in range(8):
                            w3t = wp3.tile(
                                [128, 4, D], bf16, tag="w3", name=f"w3_{ns}_{sg}_{hg}"
                            )
                            nc.sync.dma_start(w3t[:], w3b[hg, :, :, :])
                            for hj in range(4):
                                hc = hg * 4 + hj
                                for j in range(2):
                                    srel = sg * 2 + j
                                    for dsl in range(2):
                                        nc.tensor.matmul(
                                            eo[j][:, ts(dsl, 512)],
                                            hid[:, hc, ts(srel, 128)],
                                            w3t[:, hj, ts(dsl, 512)],
                                            start=(hc == 0), stop=(hc == 31),
                                        )
                        for j in range(2):
                            sglob = ns * 4 + sg * 2 + j
                            for k in range(2):
                                sc = scp.tile(
                                    [128, D], f32, tag="sc", name=f"sc_{sglob}_{k}"
                                )
                                nc.scalar.activation(
                                    sc[:], eo[j][:], AF.Copy,
                                    scale=gatesf[k][:, sglob : sglob + 1],
                                )
                                nidx = nc.gpsimd.value_load(
                                    ntile[k : k + 1, sglob : sglob + 1]
                                )
                                nc.gpsimd.dma_scatter_add(
                                    partial[:, :],
                                    sc[:].rearrange("p (o d) -> p o d", o=1),
                                    slw[k][:, sglob * 8 : (sglob + 1) * 8],
                                    num_idxs=128, num_idxs_reg=nidx, elem_size=D,
                                )

            # ================= reduce-scatter + output =================
            nc.gpsimd.collective_compute(
                "ReduceScatter", OP.add, RG, ins=[partial.opt()], outs=[rs_out.opt()]
            )
            nc.sync.dma_start(out, rs_out[:])

    nc.compile()
    return nc


def make_in_maps(x, norm_w, gate_w, w1, w2, w3):
    import ml_dtypes

    bf16 = ml_dtypes.bfloat16
    x = np.asarray(x, np.float32)
    norm_w = np.asarray(norm_w, np.float32)
    gate_w = np.asarray(gate_w, np.float32)
    w1 = np.asarray(w1, np.float32)
    w2 = np.asarray(w2, np.float32)
    w3 = np.asarray(w3, np.float32)

    xf = x.reshape(T, D)
    gweff = np.ascontiguousarray((gate_w * norm_w[None, :]).T)  # (D, E)
    ident = np.eye(128, dtype=np.float32)
    in_maps = []
    for c in range(8):
        w1e = (w1[c] * norm_w[:, None]).astype(bf16)
        w2e = (w2[c] * norm_w[:, None]).astype(bf16)
        w1s = np.ascontiguousarray(w1e.reshape(8, 128, 32, 128).transpose(1, 2, 0, 3))
        w2s = np.ascontiguousarray(w2e.reshape(8, 128, 32, 128).transpose(1, 2, 0, 3))
        w3s = np.ascontiguousarray(
            w3[c].astype(bf16).reshape(8, 4, 128, D).transpose(0, 2, 1, 3)
        )
        in_maps.append(
            {
                "xs": np.ascontiguousarray(xf[c * TPC : (c + 1) * TPC]),
                "gw": gweff,
                "w1b": w1s,
                "w2b": w2s,
                "w3b": w3s,
                "eid": np.full((16, 1), float(c), np.float32),
                "ident": ident,
            }
        )
    return in_maps


_NC = None


def _get_nc():
    global _NC
    if _NC is None:
        _NC = build_bass()
    return _NC


def run(x, norm_w, gate_w, w1, w2, w3, trace=False):
    from concourse.bass_utils import run_bass_kernel_spmd

    nc = _get_nc()
    in_maps = make_in_maps(x, norm_w, gate_w, w1, w2, w3)
    res = run_bass_kernel_spmd(nc, in_maps, core_ids=list(range(8)), trace=trace)
    outs = [res.results[c]["out"] for c in range(8)]
    full = np.concatenate(outs, axis=0).reshape(B, S, D).astype(np.float32)
    return full, res


def kernel(x, norm_w, gate_w, w1, w2, w3):
    full, _ = run(x, norm_w, gate_w, w1, w2, w3)
    return full



# revision 8
# speedup vs baseline: 1.5417x; 1.5417x over previous
"""MoE FFN (capacity-routed, top-2, SwiGLU) on 8 TRN2 NeuronCores.

Expert-parallel, one expert per core.  v2 design:
  - x is replicated (bf16 padded rows) so every core RMSNorms all T tokens
    locally -- the big xn AllGather is gone.  The router (f32, exact top-2
    selection) runs on each core's 512-token shard; only the tiny top-k
    result (idx + gates, 4x512 f32) is AllGathered.
  - Count-aware capacity: only NT*128 slots are gathered/computed/scattered
    (NT chosen on host from the actual routing, with margin; device clamps
    its counts to NT*128 so any input stays correct).  The reference's 1536
    capacity slots are ~36% occupied for typical routing.
  - FFN streams each weight byte exactly once: phase A (w1,w2 -> hidden,
    SwiGLU) over all slot slices, then phase B (w3) accumulating expert
    outputs per 512-col half of D.
  - Combine: per-slot gate scale -> dma_scatter_add of disjoint token rows
    into a bf16 (T,D) partial, then one bf16 ReduceScatter.
"""

import numpy as np

E, K, D, H = 8, 2, 1024, 4096
B, S = 2, 2048
T = B * S                      # 4096
TPC = T // 8                   # 512 tokens per core
CAP = int(1.5 * T * K / E)     # 1536
RMS_EPS = 1e-6
ROW = 1152                     # padded xn row: 1024 x | p0 | p1 | pad  (2304B % 256 == 0)


def host_routing_ntiles(x, norm_w, gate_w):
    """Replicate device routing on host to size the slot loop (NT)."""
    xf = np.asarray(x, np.float32).reshape(T, D)
    ms = np.mean(xf * xf, axis=1, keepdims=True, dtype=np.float32)
    xn = (xf / np.sqrt(ms + RMS_EPS)) * np.asarray(norm_w, np.float32)
    logits = xn @ (np.asarray(gate_w, np.float32) * np.asarray(norm_w, np.float32)).T
    top1 = logits.argmax(-1)
    l2 = logits.copy()
    l2[np.arange(T), top1] = -np.inf
    top2 = l2.argmax(-1)
    c0 = np.bincount(top1, minlength=E)
    c1 = np.bincount(top2, minlength=E)
    max_cnt = int(np.maximum(c0, c1).max())
    nt = (min(CAP, max_cnt + 32) + 127) // 128
    return max(1, nt)


def build_bass(NT):
    import concourse.bass as bass
    import concourse.mybir as mybir
    from concourse import bacc, tile

    NSLOT = NT * 128
    SCH = [(s, min(512, NSLOT - s)) for s in range(0, NSLOT, 512)]

    f32 = mybir.dt.float32
    bf16 = mybir.dt.bfloat16
    i16 = mybir.dt.int16
    i32 = mybir.dt.int32
    u32 = mybir.dt.uint32
    AF = mybir.ActivationFunctionType
    OP = mybir.AluOpType
    AX = mybir.AxisListType
    ts = bass.ts

    nc = bacc.Bacc("TRN2", target_bir_lowering=False, debug=False, num_devices=8)

    xs = nc.dram_tensor("xs", [TPC, D], f32, kind="ExternalInput").ap()
    xpad = nc.dram_tensor("xpad", [T, ROW], bf16, kind="ExternalInput").ap()
    gw = nc.dram_tensor("gw", [D, E], f32, kind="ExternalInput").ap()
    w1b = nc.dram_tensor("w1b", [128, 32, 8, 128], bf16, kind="ExternalInput").ap()
    w2b = nc.dram_tensor("w2b", [128, 32, 8, 128], bf16, kind="ExternalInput").ap()
    w3b = nc.dram_tensor("w3b", [8, 128, 4, D], bf16, kind="ExternalInput").ap()
    eid = nc.dram_tensor("eid", [16, 1], f32, kind="ExternalInput").ap()
    ident = nc.dram_tensor("ident", [128, 128], f32, kind="ExternalInput").ap()
    out = nc.dram_tensor("out", [TPC, D], bf16, kind="ExternalOutput").ap()

    RG = [list(range(8))]

    with tile.TileContext(nc) as tc:
        with (
            tc.tile_pool(name="dram", bufs=1, space="DRAM") as dp,
            tc.tile_pool(name="const", bufs=1) as cst,
            tc.tile_pool(name="lists", bufs=1) as lp,
            tc.tile_pool(name="eip", bufs=1) as eip,
        ):
            # ---- internal DRAM ----
            xn_dram = dp.tile([T + 16, ROW], bf16)
            tk_loc = dp.tile([4, TPC], f32)
            tk_full = dp.tile([8, 4, TPC], f32, addr_space="Shared")
            partial = dp.tile([T, D], bf16)
            rs_out = dp.tile([TPC, D], bf16)
            sl_dram = dp.tile([2, NSLOT], i16)
            gl_dram = dp.tile([2, NSLOT], i16)
            gate_dram = dp.tile([2, NSLOT], bf16)

            # ---- constants ----
            id_sb = cst.tile([128, 128], f32)
            nc.sync.dma_start(id_sb[:], ident)
            gw_sb = cst.tile([128, 8, E], f32)
            nc.sync.dma_start(gw_sb[:], gw.rearrange("(dc p) e -> p dc e", p=128))
            eid_sb = cst.tile([16, 1], f32)
            nc.sync.dma_start(eid_sb[:], eid)
            eps_col = cst.tile([128, 1], f32)
            nc.vector.memset(eps_col[:], RMS_EPS)

            # ---- zero-fill partial (bf16) + xn zero pad rows ----
            zf = cst.tile([128, D], bf16)
            nc.vector.memset(zf[:], 0.0)
            for i in range(T // 128):
                nc.gpsimd.dma_start(partial[ts(i, 128), :], zf[:])
            zpad = cst.tile([16, ROW], bf16)
            nc.vector.memset(zpad[:], 0.0)
            nc.gpsimd.dma_start(xn_dram[T : T + 16, :], zpad[:])

            # ---- long-lived small tiles ----
            slw = [lp.tile([128, NSLOT // 16], i16, name=f"slw{k}") for k in range(2)]
            glw = [lp.tile([128, NSLOT // 16], i16, name=f"glw{k}") for k in range(2)]
            ntile = lp.tile([2, NT], i32, name="ntile")
            gatesf = lp.tile([128, 2, NT], f32, name="gatesf")
            ei = eip.tile([128, 8, NSLOT], bf16)
            hid = eip.tile([128, 32, NSLOT], bf16)

            # ================= router (local 512 tokens, f32) =================
            with (
                tc.tile_pool(name="rout", bufs=2) as rp,
                tc.tile_pool(name="routc", bufs=4) as rc,
                tc.tile_pool(name="rpsum", bufs=2, space="PSUM") as rps,
            ):
                for i in range(TPC // 128):
                    xt = rp.tile([128, D], f32, tag="xt")
                    nc.sync.dma_start(xt[:], xs[ts(i, 128), :])
                    sq = rps.tile([128, D], f32, tag="sq")
                    ssum = rc.tile([128, 1], f32, tag="ssum")
                    nc.scalar.activation(sq[:], xt[:], AF.Square, accum_out=ssum[:])
                    s1 = rc.tile([128, 1], f32, tag="s1")
                    nc.scalar.activation(
                        s1[:], ssum[:], AF.Sqrt, bias=eps_col[:], scale=1.0 / D
                    )
                    r1 = rc.tile([128, 1], f32, tag="r1")
                    nc.vector.reciprocal(r1[:], s1[:])
                    xnf = rp.tile([128, D], f32, tag="xnf")
                    nc.scalar.activation(xnf[:], xt[:], AF.Copy, scale=r1[:])

                    # transpose x_norm tile, then logits = xnT.T @ gw -> (tok, E)
                    xnT = rp.tile([128, 8, 128], f32, tag="xnT")
                    for dc in range(8):
                        tp = rps.tile([128, 128], f32, tag="tp")
                        nc.tensor.transpose(tp[:], xnf[:, ts(dc, 128)], id_sb[:])
                        nc.scalar.copy(xnT[:, dc, :], tp[:])
                    lps = rps.tile([128, E], f32, tag="lps")
                    for dc in range(8):
                        nc.tensor.matmul(
                            lps[:], xnT[:, dc, :], gw_sb[:, dc, :],
                            start=(dc == 0), stop=(dc == 7),
                        )
                    lg = rp.tile([128, E], f32, tag="lg")
                    nc.vector.tensor_copy(lg[:], lps[:])

                    mx = rp.tile([128, 8], f32, tag="mx")
                    nc.vector.max(mx[:], lg[:])
                    mi = rp.tile([128, 8], u32, tag="mi")
                    nc.vector.max_index(mi[:], mx[:], lg[:])

                    negm1 = rc.tile([128, 1], f32, tag="negm1")
                    nc.vector.tensor_scalar_mul(negm1[:], mx[:, 0:1], -1.0)
                    ex = rp.tile([128, E], f32, tag="ex")
                    nc.scalar.activation(ex[:], lg[:], AF.Exp, bias=negm1[:])
                    zz = rc.tile([128, 1], f32, tag="zz")
                    nc.vector.reduce_sum(zz[:], ex[:], axis=AX.X)
                    t2 = rc.tile([128, 1], f32, tag="t2")
                    nc.scalar.activation(t2[:], mx[:, 1:2], AF.Exp, bias=negm1[:])
                    u0 = rc.tile([128, 1], f32, tag="u0")
                    nc.vector.scalar_tensor_tensor(
                        u0[:], zz[:], 1e-10, t2[:], op0=OP.mult, op1=OP.add
                    )
                    u1 = rc.tile([128, 1], f32, tag="u1")
                    nc.vector.tensor_scalar_add(u1[:], u0[:], 1.0)
                    p1 = rc.tile([128, 1], f32, tag="p1")
                    nc.vector.reciprocal(p1[:], u1[:])
                    p2 = rc.tile([128, 1], f32, tag="p2")
                    nc.vector.tensor_mul(p2[:], t2[:], p1[:])

                    idxf = rp.tile([128, 2], f32, tag="idxf")
                    nc.vector.tensor_copy(idxf[:], mi[:, 0:2])
                    nc.scalar.dma_start(tk_loc[0:1, ts(i, 128)], idxf[:, 0:1])
                    nc.scalar.dma_start(tk_loc[1:2, ts(i, 128)], idxf[:, 1:2])
                    nc.scalar.dma_start(tk_loc[2:3, ts(i, 128)], p1[:])
                    nc.scalar.dma_start(tk_loc[3:4, ts(i, 128)], p2[:])

            # ================= tiny all-gather of routing results =============
            nc.gpsimd.collective_compute(
                "AllGather", OP.bypass, RG, ins=[tk_loc.opt()], outs=[tk_full.opt()]
            )

            # ================= replicated RMSNorm of all T tokens =============
            # (independent of the AllGather; fills the collective latency)
            with (
                tc.tile_pool(name="nrm", bufs=3) as npo,
                tc.tile_pool(name="nrmc", bufs=3) as npc,
            ):
                for i in range(T // 128):
                    xp = npo.tile([128, ROW], bf16, tag="xp")
                    eng = nc.sync if i % 2 == 0 else nc.scalar
                    eng.dma_start(xp[:], xpad[ts(i, 128), :])
                    sqn = npo.tile([128, ROW], bf16, tag="sqn")
                    ssn = npc.tile([128, 1], f32, tag="ssn")
                    nc.scalar.activation(sqn[:], xp[:], AF.Square, accum_out=ssn[:])
                    s1n = npc.tile([128, 1], f32, tag="s1n")
                    nc.scalar.activation(
                        s1n[:], ssn[:], AF.Sqrt, bias=eps_col[:], scale=1.0 / D
                    )
                    r1n = npc.tile([128, 1], f32, tag="r1n")
                    nc.vector.reciprocal(r1n[:], s1n[:])
                    xnb = npo.tile([128, ROW], bf16, tag="xnb")
                    nc.vector.tensor_scalar(
                        out=xnb[:], in0=xp[:], scalar1=r1n[:], scalar2=None,
                        op0=OP.mult,
                    )
                    eng2 = nc.scalar if i % 2 == 0 else nc.sync
                    eng2.dma_start(xn_dram[ts(i, 128), :], xnb[:])

            # ================= positions / slot lists / gates =================
            with tc.tile_pool(name="comp", bufs=1) as cp:
                idxr = cp.tile([16, T], f32)
                gtsf = cp.tile([16, T], f32)
                for b in range(8):
                    eng = nc.sync if b % 2 == 0 else nc.scalar
                    eng.dma_start(
                        idxr[2 * b : 2 * b + 2, :],
                        tk_full[:, 0:2, :].rearrange("r f t -> f r t"),
                    )
                    eng.dma_start(
                        gtsf[2 * b : 2 * b + 2, :],
                        tk_full[:, 2:4, :].rearrange("r f t -> f r t"),
                    )
                gts = cp.tile([16, T], bf16)
                nc.vector.tensor_copy(gts[:], gtsf[:])
                mask = cp.tile([16, T], f32)
                nc.vector.tensor_scalar(
                    out=mask[:], in0=idxr[:], scalar1=eid_sb[:], scalar2=None,
                    op0=OP.is_equal,
                )
                zer16 = cp.tile([16, T], f32)
                nc.vector.memset(zer16[:], 0.0)
                cum = cp.tile([16, T], f32)
                nc.vector.tensor_tensor_scan(
                    cum[:], mask[:], zer16[:], 0.0, op0=OP.add, op1=OP.add
                )
                # per-(stream, slot-tile) valid counts for scatter descriptors
                cnt = cp.tile([2, 1], f32)
                nc.vector.reduce_sum(cnt[:], mask[0:2, :], axis=AX.X)
                nc.vector.tensor_scalar_min(cnt[:], cnt[:], float(NSLOT))
                srow = cp.tile([2, NT], f32)
                nc.gpsimd.iota(
                    srow[:], pattern=[[-128, NT]], base=0, channel_multiplier=0,
                    allow_small_or_imprecise_dtypes=True,
                )
                ntf = cp.tile([2, NT], f32)
                nc.vector.tensor_scalar(
                    out=ntf[:], in0=srow[:], scalar1=cnt[:], scalar2=None,
                    op0=OP.add,
                )
                nc.vector.tensor_scalar_min(ntf[:], ntf[:], 128.0)
                nc.vector.tensor_scalar_max(ntf[:], ntf[:], 0.0)
                nc.vector.tensor_copy(ntile[:], ntf[:])

                nc.vector.tensor_tensor(
                    out=cum[:], in0=cum[:], in1=mask[:], op=OP.mult
                )
                pos16 = cp.tile([16, T], i16)
                nc.vector.tensor_scalar(
                    out=pos16[:], in0=cum[:], scalar1=-1.0, scalar2=None,
                    op0=OP.add,
                )
                tok16 = cp.tile([16, T], i16)
                nc.gpsimd.iota(
                    tok16[:], pattern=[[1, T]], base=1, channel_multiplier=0
                )
                sraw = cp.tile([16, 2046], i16)
                nc.gpsimd.local_scatter(
                    sraw[:], tok16[:], pos16[:], channels=16, num_elems=2046,
                    num_idxs=T,
                )
                graw = cp.tile([16, 2046], bf16)
                nc.gpsimd.local_scatter(
                    graw[:], gts[:], pos16[:], channels=16, num_elems=2046,
                    num_idxs=T,
                )
                # scatter list: token idx, -1 when empty (scatter ignores)
                sl = cp.tile([16, NSLOT], i16)
                nc.vector.tensor_scalar(
                    out=sl[:], in0=sraw[:, 0:NSLOT], scalar1=-1, scalar2=None,
                    op0=OP.add,
                )
                # gather list: token idx, T (zero row) when empty
                em = cp.tile([16, NSLOT], i16)
                nc.vector.tensor_scalar(
                    out=em[:], in0=sraw[:, 0:NSLOT], scalar1=0, scalar2=None,
                    op0=OP.is_equal,
                )
                gl = cp.tile([16, NSLOT], i16)
                nc.vector.scalar_tensor_tensor(
                    gl[:], em[:], T + 1, sl[:], op0=OP.mult, op1=OP.add
                )
                nc.sync.dma_start(sl_dram[:, :], sl[0:2, :])
                nc.scalar.dma_start(gl_dram[:, :], gl[0:2, :])
                nc.gpsimd.dma_start(gate_dram[:, :], graw[0:2, 0:NSLOT])
                for k in range(2):
                    for b in range(8):
                        eng = nc.sync if b % 2 == 0 else nc.scalar
                        eng.dma_start(
                            slw[k][16 * b : 16 * (b + 1), :],
                            sl_dram[k, :].rearrange("(f p) -> p f", p=16),
                        )
                        eng.dma_start(
                            glw[k][16 * b : 16 * (b + 1), :],
                            gl_dram[k, :].rearrange("(f p) -> p f", p=16),
                        )
                gatesw = cp.tile([128, 2, NT], bf16)
                nc.sync.dma_start(
                    gatesw[:],
                    gate_dram[:, :].rearrange("k (f p) -> p k f", p=128),
                )
                nc.vector.tensor_copy(gatesf[:], gatesw[:])

            # ================= token gather =================
            with tc.tile_pool(name="gath", bufs=2) as gp:
                for s0, sw in SCH:
                    gc = []
                    for k in range(2):
                        g = gp.tile(
                            [128, 9, sw], bf16, tag=f"g{k}", name=f"g{k}_{s0}"
                        )
                        nc.gpsimd.dma_gather(
                            g[:], xn_dram[:, :],
                            glw[k][:, s0 // 16 : (s0 + sw) // 16],
                            num_idxs=sw, num_idxs_reg=sw, elem_size=ROW,
                            transpose=True,
                        )
                        gc.append(g)
                    nc.vector.tensor_tensor(
                        out=ei[:, :, s0 : s0 + sw], in0=gc[0][:, 0:8, :],
                        in1=gc[1][:, 0:8, :], op=OP.add,
                    )

            # ================= expert FFN phase A: w1/w2 + SwiGLU ============
            with (
                tc.tile_pool(name="wts12", bufs=2) as wp,
                tc.tile_pool(name="silp", bufs=2) as sp,
                tc.tile_pool(name="psA", bufs=2, space="PSUM") as ppa,
            ):
                for mg in range(8):
                    w1t = wp.tile([128, 4, 8, 128], bf16, tag="w1", name=f"w1_{mg}")
                    nc.sync.dma_start(w1t[:], w1b[:, mg * 4 : (mg + 1) * 4, :, :])
                    w2t = wp.tile([128, 4, 8, 128], bf16, tag="w2", name=f"w2_{mg}")
                    nc.sync.dma_start(w2t[:], w2b[:, mg * 4 : (mg + 1) * 4, :, :])
                    for mj in range(4):
                        m = mg * 4 + mj
                        for s0, sw in SCH:
                            ph1 = ppa.tile(
                                [128, sw], f32, tag="ph1", name=f"ph1_{m}_{s0}"
                            )
                            ph2 = ppa.tile(
                                [128, sw], f32, tag="ph2", name=f"ph2_{m}_{s0}"
                            )
                            for dc in range(8):
                                nc.tensor.matmul(
                                    ph1[:], w1t[:, mj, dc, :],
                                    ei[:, dc, s0 : s0 + sw],
                                    start=(dc == 0), stop=(dc == 7),
                                )
                            for dc in range(8):
                                nc.tensor.matmul(
                                    ph2[:], w2t[:, mj, dc, :],
                                    ei[:, dc, s0 : s0 + sw],
                                    start=(dc == 0), stop=(dc == 7),
                                )
                            slt = sp.tile(
                                [128, sw], bf16, tag="slt", name=f"sl_{m}_{s0}"
                            )
                            nc.scalar.activation(slt[:], ph1[:], AF.Sigmoid)
                            tt = sp.tile(
                                [128, sw], bf16, tag="tt", name=f"tt_{m}_{s0}"
                            )
                            nc.vector.tensor_mul(tt[:], slt[:], ph1[:])
                            nc.vector.tensor_mul(
                                hid[:, m, s0 : s0 + sw], tt[:], ph2[:]
                            )

            # ================= phase B: w3 + gates + scatter =================
            GB = NT if NT <= 6 else 4
            with (
                tc.tile_pool(name="wts3", bufs=2) as wp3,
                tc.tile_pool(name="scp", bufs=2) as scp,
                tc.tile_pool(name="psB", bufs=min(8, 2 * GB), space="PSUM") as ppb,
            ):
                for t0 in range(0, NT, GB):
                    tg = list(range(t0, min(t0 + GB, NT)))
                    sc = {
                        (t, k): scp.tile(
                            [128, D], bf16, tag=f"sc{k}", name=f"sc_{t}_{k}"
                        )
                        for t in tg
                        for k in range(2)
                    }
                    for dh in range(2):
                        eo = {
                            t: ppb.tile(
                                [128, 512], f32, tag="eo", name=f"eo_{dh}_{t}"
                            )
                            for t in tg
                        }
                        for hg in range(8):
                            w3t = wp3.tile(
                                [128, 4, 512], bf16, tag="w3",
                                name=f"w3_{t0}_{dh}_{hg}",
                            )
                            nc.sync.dma_start(
                                w3t[:], w3b[hg, :, :, ts(dh, 512)]
                            )
                            for hj in range(4):
                                hc = hg * 4 + hj
                                for t in tg:
                                    nc.tensor.matmul(
                                        eo[t][:], hid[:, hc, ts(t, 128)],
                                        w3t[:, hj, :],
                                        start=(hc == 0), stop=(hc == 31),
                                    )
                        for t in tg:
                            for k in range(2):
                                nc.scalar.activation(
                                    sc[(t, k)][:, ts(dh, 512)], eo[t][:],
                                    AF.Copy, scale=gatesf[:, k, t : t + 1],
                                )
                    for t in tg:
                        for k in range(2):
                            nidx = nc.gpsimd.value_load(
                                ntile[k : k + 1, t : t + 1]
                            )
                            nc.gpsimd.dma_scatter_add(
                                partial[:, :],
                                sc[(t, k)][:].rearrange("p (o d) -> p o d", o=1),
                                slw[k][:, t * 8 : (t + 1) * 8],
                                num_idxs=128, num_idxs_reg=nidx, elem_size=D,
                            )

            # ================= reduce-scatter + output =================
            nc.gpsimd.collective_compute(
                "ReduceScatter", OP.add, RG, ins=[partial.opt()], outs=[rs_out.opt()]
            )
            nc.sync.dma_start(out, rs_out[:])

    nc.compile()
    return nc


def make_in_maps(x, norm_w, gate_w, w1, w2, w3):
    import ml_dtypes

    bf16 = ml_dtypes.bfloat16
    x = np.asarray(x, np.float32)
    norm_w = np.asarray(norm_w, np.float32)
    gate_w = np.asarray(gate_w, np.float32)
    w1 = np.asarray(w1, np.float32)
    w2 = np.asarray(w2, np.float32)
    w3 = np.asarray(w3, np.float32)

    xf = x.reshape(T, D)
    xpad = np.zeros((T, ROW), dtype=bf16)
    xpad[:, :D] = xf.astype(bf16)
    gweff = np.ascontiguousarray((gate_w * norm_w[None, :]).T)  # (D, E)
    ident = np.eye(128, dtype=np.float32)
    in_maps = []
    for c in range(8):
        w1e = (w1[c] * norm_w[:, None]).astype(bf16)
        w2e = (w2[c] * norm_w[:, None]).astype(bf16)
        w1s = np.ascontiguousarray(w1e.reshape(8, 128, 32, 128).transpose(1, 2, 0, 3))
        w2s = np.ascontiguousarray(w2e.reshape(8, 128, 32, 128).transpose(1, 2, 0, 3))
        w3s = np.ascontiguousarray(
            w3[c].astype(bf16).reshape(8, 4, 128, D).transpose(0, 2, 1, 3)
        )
        in_maps.append(
            {
                "xs": np.ascontiguousarray(xf[c * TPC : (c + 1) * TPC]),
                "xpad": xpad,
                "gw": gweff,
                "w1b": w1s,
                "w2b": w2s,
                "w3b": w3s,
                "eid": np.full((16, 1), float(c), np.float32),
                "ident": ident,
            }
        )
    return in_maps


_NC = None
_NC_NT = None


def _get_nc(nt=5):
    global _NC, _NC_NT
    if _NC is None or _NC_NT != nt:
        _NC = build_bass(nt)
        _NC_NT = nt
    return _NC


def run(x, norm_w, gate_w, w1, w2, w3, trace=False):
    from concourse.bass_utils import run_bass_kernel_spmd

    nt = host_routing_ntiles(x, norm_w, gate_w)
    nc = _get_nc(nt)
    in_maps = make_in_maps(x, norm_w, gate_w, w1, w2, w3)
    res = run_bass_kernel_spmd(nc, in_maps, core_ids=list(range(8)), trace=trace)
    outs = [res.results[c]["out"] for c in range(8)]
    full = (
        np.concatenate(outs, axis=0).astype(np.float32).reshape(B, S, D)
    )
    return full, res


def kernel(x, norm_w, gate_w, w1, w2, w3):
    full, _ = run(x, norm_w, gate_w, w1, w2, w3)
    return full


# revision 14
# speedup vs baseline: 1.5668x; 1.0163x over previous
"""MoE FFN (capacity-routed, top-2, SwiGLU) on 8 TRN2 NeuronCores.

Expert-parallel, one expert per core.  v3 design:
  - x replicated (bf16 padded rows): every core RMSNorms all T tokens
    locally; no xn AllGather.  Router (f32, exact top-2) runs on the
    512-token shard; only idx+gates (4x512 f32) are AllGathered.
  - Count-aware capacity: NT*128 slots (NT from host routing + margin;
    device clamps counts so any input stays correct).
  - Positions via (128,64)-block cumsum + matmul prefix-sum against
    host-shipped triangular/group constants; slot lists via one gpsimd
    local_scatter; gate lists deferred off the critical path.
  - FFN: phase A (w1,w2 -> SwiGLU hidden) streams each weight byte once;
    phase B (w3) per 512-col half of D, gate-scaled outputs scattered
    per-half into two bf16 partials; two ReduceScatters (first overlaps
    the second half's compute).
"""

import numpy as np

E, K, D, H = 8, 2, 1024, 4096
B, S = 2, 2048
T = B * S                      # 4096
TPC = T // 8                   # 512 tokens per core
CAP = int(1.5 * T * K / E)     # 1536
RMS_EPS = 1e-6
ROW = 1152                     # padded xn row: 1024 x | p0 | p1 | pad  (2304B % 256 == 0)


def host_routing_ntiles(x, norm_w, gate_w):
    """Replicate device routing on host to size the slot loop (NT)."""
    xf = np.asarray(x, np.float32).reshape(T, D)
    ms = np.mean(xf * xf, axis=1, keepdims=True, dtype=np.float32)
    xn = (xf / np.sqrt(ms + RMS_EPS)) * np.asarray(norm_w, np.float32)
    logits = xn @ (np.asarray(gate_w, np.float32) * np.asarray(norm_w, np.float32)).T
    top1 = logits.argmax(-1)
    l2 = logits.copy()
    l2[np.arange(T), top1] = -np.inf
    top2 = l2.argmax(-1)
    c0 = np.bincount(top1, minlength=E)
    c1 = np.bincount(top2, minlength=E)
    max_cnt = int(np.maximum(c0, c1).max())
    nt = (min(CAP, max_cnt + 32) + 127) // 128
    return max(1, nt)


def build_bass(NT):
    import concourse.bass as bass
    import concourse.mybir as mybir
    from concourse import bacc, tile

    NSLOT = NT * 128
    SCH = [(s, min(512, NSLOT - s)) for s in range(0, NSLOT, 512)]

    f32 = mybir.dt.float32
    bf16 = mybir.dt.bfloat16
    i16 = mybir.dt.int16
    i32 = mybir.dt.int32
    u32 = mybir.dt.uint32
    AF = mybir.ActivationFunctionType
    OP = mybir.AluOpType
    AX = mybir.AxisListType
    ts = bass.ts

    nc = bacc.Bacc("TRN2", target_bir_lowering=False, debug=False, num_devices=8)

    xs = nc.dram_tensor("xs", [TPC, D], f32, kind="ExternalInput").ap()
    xpad = nc.dram_tensor("xpad", [T, ROW], bf16, kind="ExternalInput").ap()
    gw = nc.dram_tensor("gw", [D, E], f32, kind="ExternalInput").ap()
    w1b = nc.dram_tensor("w1b", [128, 32, 8, 128], bf16, kind="ExternalInput").ap()
    w2b = nc.dram_tensor("w2b", [128, 32, 8, 128], bf16, kind="ExternalInput").ap()
    w3b = nc.dram_tensor("w3b", [8, 128, 4, D], bf16, kind="ExternalInput").ap()
    eid = nc.dram_tensor("eid", [128, 1], f32, kind="ExternalInput").ap()
    ident = nc.dram_tensor("ident", [128, 128], f32, kind="ExternalInput").ap()
    ltg = nc.dram_tensor("ltg", [128, 2, 128], f32, kind="ExternalInput").ap()
    tok2 = nc.dram_tensor("tok2", [16, T], i16, kind="ExternalInput").ap()
    out = nc.dram_tensor("out", [TPC, D], bf16, kind="ExternalOutput").ap()

    RG = [list(range(8))]

    with tile.TileContext(nc) as tc:
        with (
            tc.tile_pool(name="dram", bufs=1, space="DRAM") as dp,
            tc.tile_pool(name="const", bufs=1) as cst,
            tc.tile_pool(name="lists", bufs=1) as lp,
            tc.tile_pool(name="eip", bufs=1) as eip,
        ):
            # ---- internal DRAM ----
            xn_dram = dp.tile([T + 16, ROW], bf16)
            tk_loc = dp.tile([4, TPC], f32)
            tk_full = dp.tile([8, 4, TPC], f32, addr_space="Shared")
            partial = [dp.tile([T, 512], bf16, name=f"partial{h}") for h in range(2)]
            rs_o = [dp.tile([TPC, 512], bf16, name=f"rs_o{h}") for h in range(2)]
            pos_dram = dp.tile([2, T], i16)
            cnt_dram = dp.tile([128, 1], f32)
            sl_dram = dp.tile([2, NSLOT], i16)
            gl_dram = dp.tile([2, NSLOT], i16)
            gate_dram = dp.tile([2, NSLOT], bf16)

            # ---- constants ----
            id_sb = cst.tile([128, 128], f32)
            nc.sync.dma_start(id_sb[:], ident)
            gw_sb = cst.tile([128, 8, E], f32)
            nc.sync.dma_start(gw_sb[:], gw.rearrange("(dc p) e -> p dc e", p=128))
            eid_sb = cst.tile([128, 1], f32)
            nc.sync.dma_start(eid_sb[:], eid)
            ltg_sb = cst.tile([128, 2, 128], f32)
            nc.sync.dma_start(ltg_sb[:], ltg)
            eps_col = cst.tile([128, 1], f32)
            nc.vector.memset(eps_col[:], RMS_EPS)

            # ---- long-lived small tiles ----
            slw = [lp.tile([128, NSLOT // 16], i16, name=f"slw{k}") for k in range(2)]
            glw = [lp.tile([128, NSLOT // 16], i16, name=f"glw{k}") for k in range(2)]
            ntile = lp.tile([2, NT], i32, name="ntile")
            gatesf = lp.tile([128, 2, NT], f32, name="gatesf")
            tok16 = lp.tile([16, T], i16, name="tok16")
            nc.gpsimd.dma_start(tok16[:, :], tok2)
            pos16 = lp.tile([16, T], i16, name="pos16")
            nc.vector.memset(pos16[:], -1)
            zerB = lp.tile([128, 64], f32, name="zerB")
            nc.vector.memset(zerB[:], 0.0)
            srow = lp.tile([2, NT], f32, name="srow")
            nc.gpsimd.iota(
                srow[:], pattern=[[-128, NT]], base=0, channel_multiplier=0,
                allow_small_or_imprecise_dtypes=True,
            )
            ei0 = eip.tile([128, 8, 512], bf16, name="ei0")
            ei1 = (
                eip.tile([128, 8, NSLOT - 512], bf16, name="ei1")
                if NSLOT > 512
                else None
            )
            hid = eip.tile([128, 32, NSLOT], bf16, name="hid")

            # ---- zero-fill partials (bf16) + xn zero pad rows ----
            zf = cst.tile([128, 2048], bf16)
            nc.vector.memset(zf[:], 0.0)
            for h in range(2):
                pr = partial[h].rearrange("(a p) d -> p a d", p=128)
                for i in range(8):
                    nc.gpsimd.dma_start(pr[:, 4 * i : 4 * (i + 1), :], zf[:])
            zpad = cst.tile([16, ROW], bf16)
            nc.vector.memset(zpad[:], 0.0)
            nc.gpsimd.dma_start(xn_dram[T : T + 16, :], zpad[:])

            # ================= router (local 512 tokens, f32) =================
            with (
                tc.tile_pool(name="rout", bufs=2) as rp,
                tc.tile_pool(name="routc", bufs=4) as rc,
                tc.tile_pool(name="rpsum", bufs=2, space="PSUM") as rps,
            ):
                for i in range(TPC // 128):
                    xt = rp.tile([128, D], f32, tag="xt")
                    nc.sync.dma_start(xt[:], xs[ts(i, 128), :])
                    sq = rps.tile([128, D], f32, tag="sq")
                    ssum = rc.tile([128, 1], f32, tag="ssum")
                    nc.scalar.activation(sq[:], xt[:], AF.Square, accum_out=ssum[:])
                    s1 = rc.tile([128, 1], f32, tag="s1")
                    nc.scalar.activation(
                        s1[:], ssum[:], AF.Sqrt, bias=eps_col[:], scale=1.0 / D
                    )
                    r1 = rc.tile([128, 1], f32, tag="r1")
                    nc.vector.reciprocal(r1[:], s1[:])
                    xnf = rp.tile([128, D], f32, tag="xnf")
                    nc.scalar.activation(xnf[:], xt[:], AF.Copy, scale=r1[:])

                    # transpose x_norm tile, then logits = xnT.T @ gw -> (tok, E)
                    xnT = rp.tile([128, 8, 128], f32, tag="xnT")
                    for dc in range(8):
                        tp = rps.tile([128, 128], f32, tag="tp")
                        nc.tensor.transpose(tp[:], xnf[:, ts(dc, 128)], id_sb[:])
                        nc.scalar.copy(xnT[:, dc, :], tp[:])
                    lps = rps.tile([128, E], f32, tag="lps")
                    for dc in range(8):
                        nc.tensor.matmul(
                            lps[:], xnT[:, dc, :], gw_sb[:, dc, :],
                            start=(dc == 0), stop=(dc == 7),
                        )
                    lg = rp.tile([128, E], f32, tag="lg")
                    nc.vector.tensor_copy(lg[:], lps[:])

                    mx = rp.tile([128, 8], f32, tag="mx")
                    nc.vector.max(mx[:], lg[:])
                    mi = rp.tile([128, 8], u32, tag="mi")
                    nc.vector.max_index(mi[:], mx[:], lg[:])

                    negm1 = rc.tile([128, 1], f32, tag="negm1")
                    nc.vector.tensor_scalar_mul(negm1[:], mx[:, 0:1], -1.0)
                    ex = rp.tile([128, E], f32, tag="ex")
                    nc.scalar.activation(ex[:], lg[:], AF.Exp, bias=negm1[:])
                    zz = rc.tile([128, 1], f32, tag="zz")
                    nc.vector.reduce_sum(zz[:], ex[:], axis=AX.X)
                    t2 = rc.tile([128, 1], f32, tag="t2")
                    nc.scalar.activation(t2[:], mx[:, 1:2], AF.Exp, bias=negm1[:])
                    u0 = rc.tile([128, 1], f32, tag="u0")
                    nc.vector.scalar_tensor_tensor(
                        u0[:], zz[:], 1e-10, t2[:], op0=OP.mult, op1=OP.add
                    )
                    u1 = rc.tile([128, 1], f32, tag="u1")
                    nc.vector.tensor_scalar_add(u1[:], u0[:], 1.0)
                    p1 = rc.tile([128, 1], f32, tag="p1")
                    nc.vector.reciprocal(p1[:], u1[:])
                    p2 = rc.tile([128, 1], f32, tag="p2")
                    nc.vector.tensor_mul(p2[:], t2[:], p1[:])

                    idxf = rp.tile([128, 2], f32, tag="idxf")
                    nc.vector.tensor_copy(idxf[:], mi[:, 0:2])
                    nc.gpsimd.dma_start(tk_loc[0:1, ts(i, 128)], idxf[:, 0:1])
                    nc.gpsimd.dma_start(tk_loc[1:2, ts(i, 128)], idxf[:, 1:2])
                    nc.gpsimd.dma_start(tk_loc[2:3, ts(i, 128)], p1[:])
                    nc.gpsimd.dma_start(tk_loc[3:4, ts(i, 128)], p2[:])

            # ================= tiny all-gather of routing results =============
            nc.gpsimd.collective_compute(
                "AllGather", OP.bypass, RG, ins=[tk_loc.opt()], outs=[tk_full.opt()]
            )

            # ================= replicated RMSNorm of all T tokens =============
            with (
                tc.tile_pool(name="nrm", bufs=3) as npo,
                tc.tile_pool(name="nrmc", bufs=3) as npc,
            ):
                for i in range(T // 128):
                    xp = npo.tile([128, ROW], bf16, tag="xp")
                    nc.sync.dma_start(xp[:], xpad[ts(i, 128), :])
                    sqn = npo.tile([128, D], bf16, tag="sqn")
                    ssn = npc.tile([128, 1], f32, tag="ssn")
                    nc.scalar.activation(
                        sqn[:], xp[:, 0:D], AF.Square, accum_out=ssn[:]
                    )
                    s1n = npc.tile([128, 1], f32, tag="s1n")
                    nc.scalar.activation(
                        s1n[:], ssn[:], AF.Sqrt, bias=eps_col[:], scale=1.0 / D
                    )
                    r1n = npc.tile([128, 1], f32, tag="r1n")
                    nc.vector.reciprocal(r1n[:], s1n[:])
                    xnb = npo.tile([128, ROW], bf16, tag="xnb")
                    nc.vector.tensor_scalar(
                        out=xnb[:], in0=xp[:], scalar1=r1n[:], scalar2=None,
                        op0=OP.mult,
                    )
                    nc.sync.dma_start(xn_dram[ts(i, 128), :], xnb[:])

            # ================= positions / slot lists =================
            # Block layout: partition p = k*64 + (t//64), free j = t%64.
            with (
                tc.tile_pool(name="comp", bufs=1) as cp,
                tc.tile_pool(name="cpsum", bufs=2, space="PSUM") as cps,
            ):
                idxrB = cp.tile([128, 64], f32)
                for k in range(2):
                    for r in range(8):
                        nc.gpsimd.dma_start(
                            idxrB[k * 64 + r * 8 : k * 64 + (r + 1) * 8, :],
                            tk_full[r, k, :].rearrange("(b j) -> b j", j=64),
                        )
                maskB = cp.tile([128, 64], f32)
                nc.vector.tensor_scalar(
                    out=maskB[:], in0=idxrB[:], scalar1=eid_sb[:], scalar2=None,
                    op0=OP.is_equal,
                )
                cumB = cp.tile([128, 64], f32)
                nc.vector.tensor_tensor_scan(
                    cumB[:], maskB[:], zerB[:], 0.0, op0=OP.add, op1=OP.add
                )
                bsB = cp.tile([128, 1], f32)
                nc.vector.reduce_sum(bsB[:], maskB[:], axis=AX.X)
                offp = cps.tile([128, 1], f32, tag="offp")
                nc.tensor.matmul(
                    offp[:], ltg_sb[:, 0, :], bsB[:], start=True, stop=True
                )
                cntp = cps.tile([128, 1], f32, tag="cntp")
                nc.tensor.matmul(
                    cntp[:], ltg_sb[:, 1, :], bsB[:], start=True, stop=True
                )
                offB = cp.tile([128, 1], f32)
                nc.vector.tensor_copy(offB[:], offp[:])
                cntB = cp.tile([128, 1], f32)
                nc.vector.tensor_scalar_min(cntB[:], cntp[:], float(NSLOT))
                nc.scalar.dma_start(cnt_dram[:, :], cntB[:])

                posf = cp.tile([128, 64], f32)
                nc.vector.tensor_scalar(
                    out=posf[:], in0=cumB[:], scalar1=offB[:], scalar2=None,
                    op0=OP.add,
                )
                nc.vector.tensor_tensor(
                    out=posf[:], in0=posf[:], in1=maskB[:], op=OP.mult
                )
                posB = cp.tile([128, 64], i16)
                nc.vector.tensor_scalar(
                    out=posB[:], in0=posf[:], scalar1=-1.0, scalar2=None,
                    op0=OP.add,
                )
                nc.scalar.dma_start(
                    pos_dram.rearrange("k (b j) -> (k b) j", j=64), posB[:]
                )
                nc.scalar.dma_start(pos16[0:2, :], pos_dram)

                sraw = cp.tile([16, 2046], i16)
                nc.gpsimd.local_scatter(
                    sraw[:], tok16[:], pos16[:], channels=16, num_elems=2046,
                    num_idxs=T,
                )
                # gather list: token idx, T (zero row) when empty
                sl = cp.tile([16, NSLOT], i16)
                nc.vector.tensor_scalar(
                    out=sl[:], in0=sraw[:, 0:NSLOT], scalar1=-1, scalar2=None,
                    op0=OP.add,
                )
                em = cp.tile([16, NSLOT], i16)
                nc.vector.tensor_scalar(
                    out=em[:], in0=sraw[:, 0:NSLOT], scalar1=0, scalar2=None,
                    op0=OP.is_equal,
                )
                gl = cp.tile([16, NSLOT], i16)
                nc.vector.scalar_tensor_tensor(
                    gl[:], em[:], T + 1, sl[:], op0=OP.mult, op1=OP.add
                )
                nc.scalar.dma_start(gl_dram[:, :], gl[0:2, :])
                for k in range(2):
                    eng = nc.scalar if k == 0 else nc.sync
                    for b in range(8):
                        eng.dma_start(
                            glw[k][16 * b : 16 * (b + 1), :],
                            gl_dram[k, :].rearrange("(f p) -> p f", p=16),
                        )

                # ---- token gather (critical path) ----
                with tc.tile_pool(name="gath", bufs=2) as gp:
                    gcs = {}
                    for si, (s0, sw) in enumerate(SCH):
                        for k in range(2):
                            g = gp.tile(
                                [128, 9, sw], bf16, tag=f"g{k}", name=f"g{k}_{s0}"
                            )
                            nc.gpsimd.dma_gather(
                                g[:], xn_dram[:, :],
                                glw[k][:, s0 // 16 : (s0 + sw) // 16],
                                num_idxs=sw, num_idxs_reg=sw, elem_size=ROW,
                                transpose=True,
                            )
                            gcs[(si, k)] = g
                        ei_t = ei0 if si == 0 else ei1
                        nc.vector.tensor_tensor(
                            out=ei_t[:, :, 0:sw], in0=gcs[(si, 0)][:, 0:8, :],
                            in1=gcs[(si, 1)][:, 0:8, :], op=OP.add,
                        )

                # ---- deferred (off critical path): gates, scatter lists ----
                gtsf = cp.tile([16, T], f32)
                nc.scalar.dma_start(
                    gtsf[0:2, :], tk_full[:, 2:4, :].rearrange("r f t -> f r t")
                )
                gts = cp.tile([16, T], bf16)
                nc.vector.memset(gts[:], 0.0)
                nc.vector.tensor_copy(gts[0:2, :], gtsf[0:2, :])
                graw = cp.tile([16, 2046], bf16)
                nc.gpsimd.local_scatter(
                    graw[:], gts[:], pos16[:], channels=16, num_elems=2046,
                    num_idxs=T,
                )
                nc.gpsimd.dma_start(gate_dram[:, :], graw[0:2, 0:NSLOT])
                gatesw = cp.tile([128, 2, NT], bf16)
                nc.scalar.dma_start(
                    gatesw[:],
                    gate_dram[:, :].rearrange("k (f p) -> p k f", p=128),
                )
                nc.vector.tensor_copy(gatesf[:], gatesw[:])

                nc.scalar.dma_start(sl_dram[:, :], sl[0:2, :])
                for k in range(2):
                    eng = nc.scalar if k == 0 else nc.sync
                    for b in range(8):
                        eng.dma_start(
                            slw[k][16 * b : 16 * (b + 1), :],
                            sl_dram[k, :].rearrange("(f p) -> p f", p=16),
                        )

                cntcol = cp.tile([2, 1], f32)
                nc.scalar.dma_start(
                    cntcol[:],
                    cnt_dram.rearrange("(a b) o -> a (b o)", b=64)[:, 0:1],
                )
                ntf = cp.tile([2, NT], f32)
                nc.vector.tensor_scalar(
                    out=ntf[:], in0=srow[:], scalar1=cntcol[:], scalar2=None,
                    op0=OP.add,
                )
                nc.vector.tensor_scalar_min(ntf[:], ntf[:], 128.0)
                nc.vector.tensor_scalar_max(ntf[:], ntf[:], 0.0)
                nc.vector.tensor_copy(ntile[:], ntf[:])

            # ================= expert FFN phase A: w1/w2 + SwiGLU ============
            with (
                tc.tile_pool(name="wts12", bufs=2) as wp,
                tc.tile_pool(name="silp", bufs=2) as sp,
                tc.tile_pool(name="psA", bufs=2, space="PSUM") as ppa,
            ):
                for mg in range(8):
                    w1t = wp.tile([128, 4, 8, 128], bf16, tag="w1", name=f"w1_{mg}")
                    nc.sync.dma_start(w1t[:], w1b[:, mg * 4 : (mg + 1) * 4, :, :])
                    w2t = wp.tile([128, 4, 8, 128], bf16, tag="w2", name=f"w2_{mg}")
                    nc.sync.dma_start(w2t[:], w2b[:, mg * 4 : (mg + 1) * 4, :, :])
                    for mj in range(4):
                        m = mg * 4 + mj
                        for si, (s0, sw) in enumerate(SCH):
                            ei_t = ei0 if si == 0 else ei1
                            ph1 = ppa.tile(
                                [128, sw], f32, tag="ph1", name=f"ph1_{m}_{s0}"
                            )
                            ph2 = ppa.tile(
                                [128, sw], f32, tag="ph2", name=f"ph2_{m}_{s0}"
                            )
                            for dc in range(8):
                                nc.tensor.matmul(
                                    ph1[:], w1t[:, mj, dc, :], ei_t[:, dc, 0:sw],
                                    start=(dc == 0), stop=(dc == 7),
                                )
                            for dc in range(8):
                                nc.tensor.matmul(
                                    ph2[:], w2t[:, mj, dc, :], ei_t[:, dc, 0:sw],
                                    start=(dc == 0), stop=(dc == 7),
                                )
                            slt = sp.tile(
                                [128, sw], bf16, tag="slt", name=f"sl_{m}_{s0}"
                            )
                            nc.scalar.activation(slt[:], ph1[:], AF.Sigmoid)
                            tt = sp.tile(
                                [128, sw], bf16, tag="tt", name=f"tt_{m}_{s0}"
                            )
                            nc.vector.tensor_mul(tt[:], slt[:], ph1[:])
                            nc.vector.tensor_mul(
                                hid[:, m, s0 : s0 + sw], tt[:], ph2[:]
                            )

            # ================= phase B: w3 + gates + scatter + RS ============
            GB = NT if NT <= 6 else 4
            nidx = {}
            for t in range(NT):
                for k in range(2):
                    nidx[(t, k)] = nc.gpsimd.value_load(ntile[k : k + 1, t : t + 1])
            with (
                tc.tile_pool(name="wts3", bufs=2) as wp3,
                tc.tile_pool(name="scp", bufs=4) as scp,
                tc.tile_pool(name="psB", bufs=min(8, 2 * GB), space="PSUM") as ppb,
            ):
                for t0 in range(0, NT, GB):
                    tg = list(range(t0, min(t0 + GB, NT)))
                    for dh in range(2):
                        eo = {
                            t: ppb.tile(
                                [128, 512], f32, tag="eo", name=f"eo_{dh}_{t}"
                            )
                            for t in tg
                        }
                        for hg in range(8):
                            w3t = wp3.tile(
                                [128, 4, 512], bf16, tag="w3",
                                name=f"w3_{t0}_{dh}_{hg}",
                            )
                            nc.sync.dma_start(
                                w3t[:], w3b[hg, :, :, ts(dh, 512)]
                            )
                            for hj in range(4):
                                hc = hg * 4 + hj
                                for t in tg:
                                    nc.tensor.matmul(
                                        eo[t][:], hid[:, hc, ts(t, 128)],
                                        w3t[:, hj, :],
                                        start=(hc == 0), stop=(hc == 31),
                                    )
                        for t in tg:
                            for k in range(2):
                                sc = scp.tile(
                                    [128, 512], bf16, tag=f"sc{k}",
                                    name=f"sc_{t}_{k}_{dh}",
                                )
                                nc.scalar.activation(
                                    sc[:], eo[t][:], AF.Copy,
                                    scale=gatesf[:, k, t : t + 1],
                                )
                                nc.gpsimd.dma_scatter_add(
                                    partial[dh][:, :],
                                    sc[:].rearrange("p (o d) -> p o d", o=1),
                                    slw[k][:, t * 8 : (t + 1) * 8],
                                    num_idxs=128, num_idxs_reg=nidx[(t, k)],
                                    elem_size=512,
                                )
                        if t0 + GB >= NT:
                            # last group: this half is complete -> RS it now
                            nc.gpsimd.collective_compute(
                                "ReduceScatter", OP.add, RG,
                                ins=[partial[dh].opt()], outs=[rs_o[dh].opt()],
                            )

            # ================= output =================
            for dh in range(2):
                nc.sync.dma_start(out[:, ts(dh, 512)], rs_o[dh][:])

    nc.compile()
    return nc


def make_in_maps(x, norm_w, gate_w, w1, w2, w3):
    import ml_dtypes

    bf16 = ml_dtypes.bfloat16
    x = np.asarray(x, np.float32)
    norm_w = np.asarray(norm_w, np.float32)
    gate_w = np.asarray(gate_w, np.float32)
    w1 = np.asarray(w1, np.float32)
    w2 = np.asarray(w2, np.float32)
    w3 = np.asarray(w3, np.float32)

    xf = x.reshape(T, D)
    xpad = np.zeros((T, ROW), dtype=bf16)
    xpad[:, :D] = xf.astype(bf16)
    gweff = np.ascontiguousarray((gate_w * norm_w[None, :]).T)  # (D, E)
    ident = np.eye(128, dtype=np.float32)
    # prefix-sum / group-sum constants for the (128,64) block layout:
    # partition p = k*64 + blk.  ltg[:,0,:] = L^T (exclusive prefix within
    # the 64-block group), ltg[:,1,:] = G (full group sum).
    p = np.arange(128)
    same = (p[:, None] // 64) == (p[None, :] // 64)
    L = (same & ((p[None, :] % 64) < (p[:, None] % 64))).astype(np.float32)
    G = same.astype(np.float32)
    ltg = np.stack([L.T, G], axis=1)  # (128, 2, 128); G symmetric
    ltg = np.ascontiguousarray(ltg)
    tok2 = np.broadcast_to(
        np.arange(1, T + 1, dtype=np.int16)[None, :], (16, T)
    ).copy()
    in_maps = []
    for c in range(8):
        w1e = (w1[c] * norm_w[:, None]).astype(bf16)
        w2e = (w2[c] * norm_w[:, None]).astype(bf16)
        w1s = np.ascontiguousarray(w1e.reshape(8, 128, 32, 128).transpose(1, 2, 0, 3))
        w2s = np.ascontiguousarray(w2e.reshape(8, 128, 32, 128).transpose(1, 2, 0, 3))
        w3s = np.ascontiguousarray(
            w3[c].astype(bf16).reshape(8, 4, 128, D).transpose(0, 2, 1, 3)
        )
        in_maps.append(
            {
                "xs": np.ascontiguousarray(xf[c * TPC : (c + 1) * TPC]),
                "xpad": xpad,
                "gw": gweff,
                "w1b": w1s,
                "w2b": w2s,
                "w3b": w3s,
                "eid": np.full((128, 1), float(c), np.float32),
                "ident": ident,
                "ltg": ltg,
                "tok2": tok2,
            }
        )
    return in_maps


_NC = None
_NC_NT = None


def _get_nc(nt=5):
    global _NC, _NC_NT
    if _NC is None or _NC_NT != nt:
        _NC = build_bass(nt)
        _NC_NT = nt
    return _NC


def run(x, norm_w, gate_w, w1, w2, w3, trace=False):
    from concourse.bass_utils import run_bass_kernel_spmd

    nt = host_routing_ntiles(x, norm_w, gate_w)
    nc = _get_nc(nt)
    in_maps = make_in_maps(x, norm_w, gate_w, w1, w2, w3)
    res = run_bass_kernel_spmd(nc, in_maps, core_ids=list(range(8)), trace=trace)
    outs = [res.results[c]["out"] for c in range(8)]
    full = (
        np.concatenate(outs, axis=0).astype(np.float32).reshape(B, S, D)
    )
    return full, res


def kernel(x, norm_w, gate_w, w1, w2, w3):
    full, _ = run(x, norm_w, gate_w, w1, w2, w3)
    return full


# revision 19
# speedup vs baseline: 1.6696x; 1.0656x over previous
"""MoE FFN (capacity-routed, top-2, SwiGLU) on 8 TRN2 NeuronCores.

Expert-parallel, one expert per core.  v3 design:
  - x replicated (bf16 padded rows): every core RMSNorms all T tokens
    locally; no xn AllGather.  Router (f32, exact top-2) runs on the
    512-token shard; only idx+gates (4x512 f32) are AllGathered.
  - Count-aware capacity: NT*128 slots (NT from host routing + margin;
    device clamps counts so any input stays correct).
  - Positions via (128,64)-block cumsum + matmul prefix-sum against
    host-shipped triangular/group constants; slot lists via one gpsimd
    local_scatter; gate lists deferred off the critical path.
  - FFN: phase A (w1,w2 -> SwiGLU hidden) streams each weight byte once;
    phase B (w3) per 512-col half of D, gate-scaled outputs scattered
    per-half into two bf16 partials; two ReduceScatters (first overlaps
    the second half's compute).
"""

import numpy as np

E, K, D, H = 8, 2, 1024, 4096
B, S = 2, 2048
T = B * S                      # 4096
TPC = T // 8                   # 512 tokens per core
CAP = int(1.5 * T * K / E)     # 1536
RMS_EPS = 1e-6
ROW = 1152                     # padded xn row: 1024 x | p0 | p1 | pad  (2304B % 256 == 0)


def host_routing_ntiles(x, norm_w, gate_w):
    """Replicate device routing on host to size the slot loop (NT)."""
    xf = np.asarray(x, np.float32).reshape(T, D)
    ms = np.mean(xf * xf, axis=1, keepdims=True, dtype=np.float32)
    xn = (xf / np.sqrt(ms + RMS_EPS)) * np.asarray(norm_w, np.float32)
    logits = xn @ (np.asarray(gate_w, np.float32) * np.asarray(norm_w, np.float32)).T
    top1 = logits.argmax(-1)
    l2 = logits.copy()
    l2[np.arange(T), top1] = -np.inf
    top2 = l2.argmax(-1)
    c0 = np.bincount(top1, minlength=E)
    c1 = np.bincount(top2, minlength=E)
    max_cnt = int(np.maximum(c0, c1).max())
    nt = (min(CAP, max_cnt + 32) + 127) // 128
    return max(1, nt)


def build_bass(NT):
    import concourse.bass as bass
    import concourse.mybir as mybir
    from concourse import bacc, tile

    NSLOT = NT * 128
    SCH = [(s, min(512, NSLOT - s)) for s in range(0, NSLOT, 512)]

    f32 = mybir.dt.float32
    bf16 = mybir.dt.bfloat16
    i16 = mybir.dt.int16
    i32 = mybir.dt.int32
    u32 = mybir.dt.uint32
    AF = mybir.ActivationFunctionType
    OP = mybir.AluOpType
    AX = mybir.AxisListType
    ts = bass.ts

    nc = bacc.Bacc("TRN2", target_bir_lowering=False, debug=False, num_devices=8)

    xs = nc.dram_tensor("xs", [TPC, D], f32, kind="ExternalInput").ap()
    xpad = nc.dram_tensor("xpad", [T, ROW], bf16, kind="ExternalInput").ap()
    gw = nc.dram_tensor("gw", [D, E], f32, kind="ExternalInput").ap()
    w1b = nc.dram_tensor("w1b", [128, 32, 8, 128], bf16, kind="ExternalInput").ap()
    w2b = nc.dram_tensor("w2b", [128, 32, 8, 128], bf16, kind="ExternalInput").ap()
    w3b = nc.dram_tensor("w3b", [8, 128, 4, D], bf16, kind="ExternalInput").ap()
    eid = nc.dram_tensor("eid", [128, 1], f32, kind="ExternalInput").ap()
    ident = nc.dram_tensor("ident", [128, 128], f32, kind="ExternalInput").ap()
    ltg = nc.dram_tensor("ltg", [128, 2, 128], f32, kind="ExternalInput").ap()
    tok2 = nc.dram_tensor("tok2", [16, T], i16, kind="ExternalInput").ap()
    out = nc.dram_tensor("out", [TPC, D], bf16, kind="ExternalOutput").ap()

    RG = [list(range(8))]

    with tile.TileContext(nc) as tc:
        with (
            tc.tile_pool(name="dram", bufs=1, space="DRAM") as dp,
            tc.tile_pool(name="const", bufs=1) as cst,
            tc.tile_pool(name="lists", bufs=1) as lp,
            tc.tile_pool(name="eip", bufs=1) as eip,
        ):
            # ---- internal DRAM ----
            xn_dram = dp.tile([T + 16, ROW], bf16)
            tk_loc = dp.tile([4, TPC], f32)
            tk_full = dp.tile([8, 4, TPC], f32, addr_space="Shared")
            partial = [dp.tile([T, 512], bf16, name=f"partial{h}") for h in range(2)]
            rs_o = [dp.tile([TPC, 512], bf16, name=f"rs_o{h}") for h in range(2)]
            pos_dram = dp.tile([2, T], i16)
            cnt_dram = dp.tile([128, 1], f32)
            sl_dram = dp.tile([2, NSLOT], i16)
            gl_dram = dp.tile([2, NSLOT], i16)
            gate_dram = dp.tile([2, NSLOT], bf16)

            # ---- constants ----
            id_sb = cst.tile([128, 128], f32)
            nc.sync.dma_start(id_sb[:], ident)
            gw_sb = cst.tile([128, 8, E], f32)
            nc.sync.dma_start(gw_sb[:], gw.rearrange("(dc p) e -> p dc e", p=128))
            eid_sb = cst.tile([128, 1], f32)
            nc.sync.dma_start(eid_sb[:], eid)
            ltg_sb = cst.tile([128, 2, 128], f32)
            nc.sync.dma_start(ltg_sb[:], ltg)
            eps_col = cst.tile([128, 1], f32)
            nc.vector.memset(eps_col[:], RMS_EPS)

            # ---- long-lived small tiles ----
            slw = [lp.tile([128, NSLOT // 16], i16, name=f"slw{k}") for k in range(2)]
            glw = [lp.tile([128, NSLOT // 16], i16, name=f"glw{k}") for k in range(2)]
            ntile = lp.tile([2, NT], i32, name="ntile")
            gatesf = lp.tile([128, 2, NT], f32, name="gatesf")
            tok16 = lp.tile([16, T], i16, name="tok16")
            nc.gpsimd.dma_start(tok16[:, :], tok2)
            pos16 = lp.tile([16, T], i16, name="pos16")
            nc.vector.memset(pos16[:], -1)
            zerB = lp.tile([128, 64], f32, name="zerB")
            nc.vector.memset(zerB[:], 0.0)
            srow = lp.tile([2, NT], f32, name="srow")
            nc.gpsimd.iota(
                srow[:], pattern=[[-128, NT]], base=0, channel_multiplier=0,
                allow_small_or_imprecise_dtypes=True,
            )
            ei0 = eip.tile([128, 8, 512], bf16, name="ei0")
            ei1 = (
                eip.tile([128, 8, NSLOT - 512], bf16, name="ei1")
                if NSLOT > 512
                else None
            )
            hid = eip.tile([128, 32, NSLOT], bf16, name="hid")

            # ---- xn zero pad rows (zero-fill of partials is emitted later,
            # off the gpsimd critical path) ----
            zf = cst.tile([128, 2048], bf16)
            nc.vector.memset(zf[:], 0.0)
            zpad = cst.tile([16, ROW], bf16)
            nc.vector.memset(zpad[:], 0.0)
            nc.gpsimd.dma_start(xn_dram[T : T + 16, :], zpad[:])

            # ================= router (local 512 tokens, f32) =================
            with (
                tc.tile_pool(name="rout", bufs=2) as rp,
                tc.tile_pool(name="routc", bufs=4) as rc,
                tc.tile_pool(name="rpsum", bufs=2, space="PSUM") as rps,
            ):
                for i in range(TPC // 128):
                    xt = rp.tile([128, D], f32, tag="xt")
                    nc.sync.dma_start(xt[:], xs[ts(i, 128), :])
                    sq = rps.tile([128, D], f32, tag="sq")
                    ssum = rc.tile([128, 1], f32, tag="ssum")
                    nc.scalar.activation(sq[:], xt[:], AF.Square, accum_out=ssum[:])
                    s1 = rc.tile([128, 1], f32, tag="s1")
                    nc.scalar.activation(
                        s1[:], ssum[:], AF.Sqrt, bias=eps_col[:], scale=1.0 / D
                    )
                    r1 = rc.tile([128, 1], f32, tag="r1")
                    nc.vector.reciprocal(r1[:], s1[:])
                    xnf = rp.tile([128, D], f32, tag="xnf")
                    nc.scalar.activation(xnf[:], xt[:], AF.Copy, scale=r1[:])

                    # transpose x_norm tile, then logits = xnT.T @ gw -> (tok, E)
                    xnT = rp.tile([128, 8, 128], f32, tag="xnT")
                    for dc in range(8):
                        tp = rps.tile([128, 128], f32, tag="tp")
                        nc.tensor.transpose(tp[:], xnf[:, ts(dc, 128)], id_sb[:])
                        nc.scalar.copy(xnT[:, dc, :], tp[:])
                    lps = rps.tile([128, E], f32, tag="lps")
                    for dc in range(8):
                        nc.tensor.matmul(
                            lps[:], xnT[:, dc, :], gw_sb[:, dc, :],
                            start=(dc == 0), stop=(dc == 7),
                        )
                    lg = rp.tile([128, E], f32, tag="lg")
                    nc.vector.tensor_copy(lg[:], lps[:])

                    mx = rp.tile([128, 8], f32, tag="mx")
                    nc.vector.max(mx[:], lg[:])
                    mi = rp.tile([128, 8], u32, tag="mi")
                    nc.vector.max_index(mi[:], mx[:], lg[:])

                    negm1 = rc.tile([128, 1], f32, tag="negm1")
                    nc.vector.tensor_scalar_mul(negm1[:], mx[:, 0:1], -1.0)
                    ex = rp.tile([128, E], f32, tag="ex")
                    nc.scalar.activation(ex[:], lg[:], AF.Exp, bias=negm1[:])
                    zz = rc.tile([128, 1], f32, tag="zz")
                    nc.vector.reduce_sum(zz[:], ex[:], axis=AX.X)
                    t2 = rc.tile([128, 1], f32, tag="t2")
                    nc.scalar.activation(t2[:], mx[:, 1:2], AF.Exp, bias=negm1[:])
                    u0 = rc.tile([128, 1], f32, tag="u0")
                    nc.vector.scalar_tensor_tensor(
                        u0[:], zz[:], 1e-10, t2[:], op0=OP.mult, op1=OP.add
                    )
                    u1 = rc.tile([128, 1], f32, tag="u1")
                    nc.vector.tensor_scalar_add(u1[:], u0[:], 1.0)
                    p1 = rc.tile([128, 1], f32, tag="p1")
                    nc.vector.reciprocal(p1[:], u1[:])
                    p2 = rc.tile([128, 1], f32, tag="p2")
                    nc.vector.tensor_mul(p2[:], t2[:], p1[:])

                    idxf = rp.tile([128, 2], f32, tag="idxf")
                    nc.vector.tensor_copy(idxf[:], mi[:, 0:2])
                    nc.gpsimd.dma_start(tk_loc[0:1, ts(i, 128)], idxf[:, 0:1])
                    nc.gpsimd.dma_start(tk_loc[1:2, ts(i, 128)], idxf[:, 1:2])
                    nc.gpsimd.dma_start(tk_loc[2:3, ts(i, 128)], p1[:])
                    nc.gpsimd.dma_start(tk_loc[3:4, ts(i, 128)], p2[:])

            # ================= tiny all-gather of routing results =============
            nc.gpsimd.collective_compute(
                "AllGather", OP.bypass, RG, ins=[tk_loc.opt()], outs=[tk_full.opt()]
            )

            # ================= replicated RMSNorm of all T tokens =============
            # 8-tile batches: 4 big loads + 4 big stores keep the DMA issue
            # count off the engines' critical paths.
            NB = 8
            xpr = xpad.rearrange("(i p) r -> p i r", p=128)
            xnr = xn_dram[0:T, :].rearrange("(i p) r -> p i r", p=128)
            with (
                tc.tile_pool(name="nrm", bufs=3) as npo,
                tc.tile_pool(name="nrms", bufs=2) as nps,
                tc.tile_pool(name="nrmc", bufs=3) as npc,
            ):
                for ib in range(T // 128 // NB):
                    xp = npo.tile([128, NB, ROW], bf16, tag="xp")
                    nc.sync.dma_start(
                        xp[:], xpr[:, ib * NB : (ib + 1) * NB, :]
                    )
                    xnb = nps.tile([128, NB, ROW], bf16, tag="xnb")
                    for j in range(NB):
                        sqn = npo.tile([128, D], bf16, tag="sqn")
                        ssn = npc.tile([128, 1], f32, tag="ssn")
                        nc.scalar.activation(
                            sqn[:], xp[:, j, 0:D], AF.Square, accum_out=ssn[:]
                        )
                        s1n = npc.tile([128, 1], f32, tag="s1n")
                        nc.scalar.activation(
                            s1n[:], ssn[:], AF.Sqrt, bias=eps_col[:],
                            scale=1.0 / D,
                        )
                        r1n = npc.tile([128, 1], f32, tag="r1n")
                        nc.vector.reciprocal(r1n[:], s1n[:])
                        nc.vector.tensor_scalar(
                            out=xnb[:, j, :], in0=xp[:, j, :], scalar1=r1n[:],
                            scalar2=None, op0=OP.mult,
                        )
                    nc.scalar.dma_start(
                        xnr[:, ib * NB : (ib + 1) * NB, :], xnb[:]
                    )

            # ================= positions / slot lists =================
            # Block layout: partition p = k*64 + (t//64), free j = t%64.
            with (
                tc.tile_pool(name="comp", bufs=1) as cp,
                tc.tile_pool(name="cpsum", bufs=2, space="PSUM") as cps,
            ):
                idxrB = cp.tile([128, 64], f32)
                for k in range(2):
                    for r in range(8):
                        nc.sync.dma_start(
                            idxrB[k * 64 + r * 8 : k * 64 + (r + 1) * 8, :],
                            tk_full[r, k, :].rearrange("(b j) -> b j", j=64),
                        )
                maskB = cp.tile([128, 64], f32)
                nc.vector.tensor_scalar(
                    out=maskB[:], in0=idxrB[:], scalar1=eid_sb[:], scalar2=None,
                    op0=OP.is_equal,
                )
                cumB = cp.tile([128, 64], f32)
                nc.vector.tensor_tensor_scan(
                    cumB[:], maskB[:], zerB[:], 0.0, op0=OP.add, op1=OP.add
                )
                bsB = cp.tile([128, 1], f32)
                nc.vector.reduce_sum(bsB[:], maskB[:], axis=AX.X)
                offp = cps.tile([128, 1], f32, tag="offp")
                nc.tensor.matmul(
                    offp[:], ltg_sb[:, 0, :], bsB[:], start=True, stop=True
                )
                cntp = cps.tile([128, 1], f32, tag="cntp")
                nc.tensor.matmul(
                    cntp[:], ltg_sb[:, 1, :], bsB[:], start=True, stop=True
                )
                offB = cp.tile([128, 1], f32)
                nc.vector.tensor_copy(offB[:], offp[:])
                cntB = cp.tile([128, 1], f32)
                nc.vector.tensor_scalar_min(cntB[:], cntp[:], float(NSLOT))
                nc.scalar.dma_start(cnt_dram[:, :], cntB[:])

                posf = cp.tile([128, 64], f32)
                nc.vector.tensor_scalar(
                    out=posf[:], in0=cumB[:], scalar1=offB[:], scalar2=None,
                    op0=OP.add,
                )
                nc.vector.tensor_tensor(
                    out=posf[:], in0=posf[:], in1=maskB[:], op=OP.mult
                )
                posB = cp.tile([128, 64], i16)
                nc.vector.tensor_scalar(
                    out=posB[:], in0=posf[:], scalar1=-1.0, scalar2=None,
                    op0=OP.add,
                )
                nc.scalar.dma_start(
                    pos_dram.rearrange("k (b j) -> (k b) j", j=64), posB[:]
                )
                nc.scalar.dma_start(pos16[0:2, :], pos_dram)

                sraw = cp.tile([16, 2046], i16)
                nc.gpsimd.local_scatter(
                    sraw[:], tok16[:], pos16[:], channels=16, num_elems=2046,
                    num_idxs=T,
                )
                # scatter list: token idx, -1 when empty; gather list: same but
                # empty -> T (zero row) via unsigned min (-1 = 0xffff -> T).
                sl = cp.tile([16, NSLOT], i16)
                nc.vector.tensor_scalar(
                    out=sl[:], in0=sraw[:, 0:NSLOT], scalar1=-1, scalar2=None,
                    op0=OP.add,
                )
                gl = cp.tile([16, NSLOT], i16)
                nc.vector.tensor_scalar_min(
                    gl[:].bitcast(mybir.dt.uint16), sl[:].bitcast(mybir.dt.uint16), T
                )
                nc.scalar.dma_start(gl_dram[:, :], gl[0:2, :])
                for k in range(2):
                    eng = nc.scalar if k == 0 else nc.sync
                    for b in range(8):
                        eng.dma_start(
                            glw[k][16 * b : 16 * (b + 1), :],
                            gl_dram[k, :].rearrange("(f p) -> p f", p=16),
                        )

                # ---- token gather (critical path) ----
                with tc.tile_pool(name="gath", bufs=2) as gp:
                    gcs = {}
                    for si, (s0, sw) in enumerate(SCH):
                        for k in range(2):
                            g = gp.tile(
                                [128, 9, sw], bf16, tag=f"g{k}", name=f"g{k}_{s0}"
                            )
                            nc.gpsimd.dma_gather(
                                g[:], xn_dram[:, :],
                                glw[k][:, s0 // 16 : (s0 + sw) // 16],
                                num_idxs=sw, num_idxs_reg=sw, elem_size=ROW,
                                transpose=True,
                            )
                            gcs[(si, k)] = g
                        ei_t = ei0 if si == 0 else ei1
                        nc.vector.tensor_tensor(
                            out=ei_t[:, :, 0:sw], in0=gcs[(si, 0)][:, 0:8, :],
                            in1=gcs[(si, 1)][:, 0:8, :], op=OP.add,
                        )

                # ---- deferred (off critical path): gates, scatter lists ----
                gtsf = cp.tile([16, T], f32)
                nc.scalar.dma_start(
                    gtsf[0:2, :], tk_full[:, 2:4, :].rearrange("r f t -> f r t")
                )
                gts = cp.tile([16, T], bf16)
                nc.vector.memset(gts[:], 0.0)
                nc.vector.tensor_copy(gts[0:2, :], gtsf[0:2, :])
                graw = cp.tile([16, 2046], bf16)
                nc.gpsimd.local_scatter(
                    graw[:], gts[:], pos16[:], channels=16, num_elems=2046,
                    num_idxs=T,
                )
                nc.gpsimd.dma_start(gate_dram[:, :], graw[0:2, 0:NSLOT])
                gatesw = cp.tile([128, 2, NT], bf16)
                nc.scalar.dma_start(
                    gatesw[:],
                    gate_dram[:, :].rearrange("k (f p) -> p k f", p=128),
                )
                nc.vector.tensor_copy(gatesf[:], gatesw[:])

                nc.scalar.dma_start(sl_dram[:, :], sl[0:2, :])
                for k in range(2):
                    eng = nc.scalar if k == 0 else nc.sync
                    for b in range(8):
                        eng.dma_start(
                            slw[k][16 * b : 16 * (b + 1), :],
                            sl_dram[k, :].rearrange("(f p) -> p f", p=16),
                        )

                # zero-fill partials now: gpsimd is off the critical path here
                # and the first scatter_add is far away.
                for h in range(2):
                    pr = partial[h].rearrange("(a p) d -> p a d", p=128)
                    for i in range(8):
                        nc.gpsimd.dma_start(pr[:, 4 * i : 4 * (i + 1), :], zf[:])

                cntcol = cp.tile([2, 1], f32)
                nc.scalar.dma_start(
                    cntcol[:],
                    cnt_dram.rearrange("(a b) o -> a (b o)", b=64)[:, 0:1],
                )
                ntf = cp.tile([2, NT], f32)
                nc.vector.tensor_scalar(
                    out=ntf[:], in0=srow[:], scalar1=cntcol[:], scalar2=None,
                    op0=OP.add,
                )
                nc.vector.tensor_scalar_min(ntf[:], ntf[:], 128.0)
                nc.vector.tensor_scalar_max(ntf[:], ntf[:], 0.0)
                nc.vector.tensor_copy(ntile[:], ntf[:])

            # ================= expert FFN phase A: w1/w2 + SwiGLU ============
            with (
                tc.tile_pool(name="wts12", bufs=2) as wp,
                tc.tile_pool(name="silp", bufs=2) as sp,
                tc.tile_pool(name="psA", bufs=2, space="PSUM") as ppa,
            ):
                for mg in range(8):
                    w1t = wp.tile([128, 4, 8, 128], bf16, tag="w1", name=f"w1_{mg}")
                    nc.sync.dma_start(w1t[:], w1b[:, mg * 4 : (mg + 1) * 4, :, :])
                    w2t = wp.tile([128, 4, 8, 128], bf16, tag="w2", name=f"w2_{mg}")
                    nc.sync.dma_start(w2t[:], w2b[:, mg * 4 : (mg + 1) * 4, :, :])
                    for mj in range(4):
                        m = mg * 4 + mj
                        for si, (s0, sw) in enumerate(SCH):
                            ei_t = ei0 if si == 0 else ei1
                            ph1 = ppa.tile(
                                [128, sw], f32, tag="ph1", name=f"ph1_{m}_{s0}"
                            )
                            ph2 = ppa.tile(
                                [128, sw], f32, tag="ph2", name=f"ph2_{m}_{s0}"
                            )
                            for dc in range(8):
                                nc.tensor.matmul(
                                    ph1[:], w1t[:, mj, dc, :], ei_t[:, dc, 0:sw],
                                    start=(dc == 0), stop=(dc == 7),
                                )
                            for dc in range(8):
                                nc.tensor.matmul(
                                    ph2[:], w2t[:, mj, dc, :], ei_t[:, dc, 0:sw],
                                    start=(dc == 0), stop=(dc == 7),
                                )
                            slt = sp.tile(
                                [128, sw], bf16, tag="slt", name=f"sl_{m}_{s0}"
                            )
                            nc.scalar.activation(slt[:], ph1[:], AF.Sigmoid)
                            tt = sp.tile(
                                [128, sw], bf16, tag="tt", name=f"tt_{m}_{s0}"
                            )
                            nc.vector.tensor_mul(tt[:], slt[:], ph1[:])
                            nc.vector.tensor_mul(
                                hid[:, m, s0 : s0 + sw], tt[:], ph2[:]
                            )

            # ================= phase B: w3 + gates + scatter + RS ============
            GB = NT if NT <= 6 else 4
            nidx = {}
            for t in range(NT):
                for k in range(2):
                    nidx[(t, k)] = nc.gpsimd.value_load(ntile[k : k + 1, t : t + 1])
            with (
                tc.tile_pool(name="wts3", bufs=2) as wp3,
                tc.tile_pool(name="scp", bufs=4) as scp,
                tc.tile_pool(name="psB", bufs=min(8, 2 * GB), space="PSUM") as ppb,
            ):
                for t0 in range(0, NT, GB):
                    tg = list(range(t0, min(t0 + GB, NT)))
                    for dh in range(2):
                        eo = {
                            t: ppb.tile(
                                [128, 512], f32, tag="eo", name=f"eo_{dh}_{t}"
                            )
                            for t in tg
                        }
                        for hg in range(8):
                            w3t = wp3.tile(
                                [128, 4, 512], bf16, tag="w3",
                                name=f"w3_{t0}_{dh}_{hg}",
                            )
                            nc.sync.dma_start(
                                w3t[:], w3b[hg, :, :, ts(dh, 512)]
                            )
                            for hj in range(4):
                                hc = hg * 4 + hj
                                for t in tg:
                                    nc.tensor.matmul(
                                        eo[t][:], hid[:, hc, ts(t, 128)],
                                        w3t[:, hj, :],
                                        start=(hc == 0), stop=(hc == 31),
                                    )
                        for t in tg:
                            for k in range(2):
                                sc = scp.tile(
                                    [128, 512], bf16, tag=f"sc{k}",
                                    name=f"sc_{t}_{k}_{dh}",
                                )
                                nc.scalar.activation(
                                    sc[:], eo[t][:], AF.Copy,
                                    scale=gatesf[:, k, t : t + 1],
                                )
                                nc.gpsimd.dma_scatter_add(
                                    partial[dh][:, :],
                                    sc[:].rearrange("p (o d) -> p o d", o=1),
                                    slw[k][:, t * 8 : (t + 1) * 8],
                                    num_idxs=128, num_idxs_reg=nidx[(t, k)],
                                    elem_size=512,
                                )
                        if t0 + GB >= NT:
                            # last group: this half is complete -> RS it now
                            nc.gpsimd.collective_compute(
                                "ReduceScatter", OP.add, RG,
                                ins=[partial[dh].opt()], outs=[rs_o[dh].opt()],
                            )

            # ================= output =================
            for dh in range(2):
                nc.sync.dma_start(out[:, ts(dh, 512)], rs_o[dh][:])

    nc.compile()
    return nc


def make_in_maps(x, norm_w, gate_w, w1, w2, w3):
    import ml_dtypes

    bf16 = ml_dtypes.bfloat16
    x = np.asarray(x, np.float32)
    norm_w = np.asarray(norm_w, np.float32)
    gate_w = np.asarray(gate_w, np.float32)
    w1 = np.asarray(w1, np.float32)
    w2 = np.asarray(w2, np.float32)
    w3 = np.asarray(w3, np.float32)

    xf = x.reshape(T, D)
    xpad = np.zeros((T, ROW), dtype=bf16)
    xpad[:, :D] = xf.astype(bf16)
    gweff = np.ascontiguousarray((gate_w * norm_w[None, :]).T)  # (D, E)
    ident = np.eye(128, dtype=np.float32)
    # prefix-sum / group-sum constants for the (128,64) block layout:
    # partition p = k*64 + blk.  ltg[:,0,:] = L^T (exclusive prefix within
    # the 64-block group), ltg[:,1,:] = G (full group sum).
    p = np.arange(128)
    same = (p[:, None] // 64) == (p[None, :] // 64)
    L = (same & ((p[None, :] % 64) < (p[:, None] % 64))).astype(np.float32)
    G = same.astype(np.float32)
    ltg = np.stack([L.T, G], axis=1)  # (128, 2, 128); G symmetric
    ltg = np.ascontiguousarray(ltg)
    tok2 = np.broadcast_to(
        np.arange(1, T + 1, dtype=np.int16)[None, :], (16, T)
    ).copy()
    in_maps = []
    for c in range(8):
        w1e = (w1[c] * norm_w[:, None]).astype(bf16)
        w2e = (w2[c] * norm_w[:, None]).astype(bf16)
        w1s = np.ascontiguousarray(w1e.reshape(8, 128, 32, 128).transpose(1, 2, 0, 3))
        w2s = np.ascontiguousarray(w2e.reshape(8, 128, 32, 128).transpose(1, 2, 0, 3))
        w3s = np.ascontiguousarray(
            w3[c].astype(bf16).reshape(8, 4, 128, D).transpose(0, 2, 1, 3)
        )
        in_maps.append(
            {
                "xs": np.ascontiguousarray(xf[c * TPC : (c + 1) * TPC]),
                "xpad": xpad,
                "gw": gweff,
                "w1b": w1s,
                "w2b": w2s,
                "w3b": w3s,
                "eid": np.full((128, 1), float(c), np.float32),
                "ident": ident,
                "ltg": ltg,
                "tok2": tok2,
            }
        )
    return in_maps


_NC = None
_NC_NT = None


def _get_nc(nt=5):
    global _NC, _NC_NT
    if _NC is None or _NC_NT != nt:
        _NC = build_bass(nt)
        _NC_NT = nt
    return _NC


def run(x, norm_w, gate_w, w1, w2, w3, trace=False):
    from concourse.bass_utils import run_bass_kernel_spmd

    nt = host_routing_ntiles(x, norm_w, gate_w)
    nc = _get_nc(nt)
    in_maps = make_in_maps(x, norm_w, gate_w, w1, w2, w3)
    res = run_bass_kernel_spmd(nc, in_maps, core_ids=list(range(8)), trace=trace)
    outs = [res.results[c]["out"] for c in range(8)]
    full = (
        np.concatenate(outs, axis=0).astype(np.float32).reshape(B, S, D)
    )
    return full, res


def kernel(x, norm_w, gate_w, w1, w2, w3):
    full, _ = run(x, norm_w, gate_w, w1, w2, w3)
    return full
